# revision 59
# baseline (speedup 1.0000x reference)
"""Trainium2 Bass kernel for nn_Block_22325240004804 (dense_transformer).

Two-stream cross-attention transformer block, B=8 N=1024 C=768 H=12.
Sharding: pure data parallel - batch element b on core b (no collectives).

v8 (on top of v5; 901us -> ~670us):
  - merged-direction attention: per (head, parity) both directions'
    QK matmuls issue as one 8-instruction burst, exps for dir1 overlap
    dir2's QKs, and PV is deferred one mc2 so it never waits on exp
    (PE p-state: only >3us continuous bursts reach max clock; the old
    per-dir chains ran everything at mid/low clock)
  - pt2 layout [P, nh, mi, n]: PV DoubleRow k-tile pair contiguous
  - phase1: LN1 chain issued one chunk ahead of its transposes; QKV
    sections aligned to q/k/v (768 each) and BOTH streams' sections
    interleaved, so ~20us of matmuls cover the per-section stats chains
    (square->reduce->sqrt->recip->normalize) running two-wide on
    vector/scalar; k-transposes stored before q so attention's first
    loads unblock sooner
  - bf16 staging for xn/o residuals (halves staging DMA, kills casts);
    xnT copies on scalar; LN2 + residual adds on vector (scalar is
    exp/gelu-critical); denominator shift DMAs on gpsimd
  - proj software-pipelined: LN2-apply/transpose stage skewed one chunk
    behind matmul/residual; mlp: fc1_a, then fc1_b || fc2_a, then fc2_b
    with kk-major fc1 so gelu(kk=0) overlaps kk=1 matmuls
  - w2 prefetched at attention start (reuses wq's SBUF slot)
  Rejected experimentally: fp8 DoubleRow QK (no DR speedup at mid
  p-state, +LDW cost), fp8 QKV (weight quantization error 2.7e-2 >
  2e-2 gate), gpsimd tensor ops on critical paths (too slow).
"""

import sys

if "/opt/trn_rl_repo" not in sys.path:
    sys.path.insert(0, "/opt/trn_rl_repo")

import numpy as np

B, N, C = 8, 1024, 768
H, HD = 12, 64
S3 = 3 * C          # 2304
HID = 4 * C         # 3072
EPS = 1e-5
P = 128
NCH = N // P        # 8 token chunks
KC = C // P         # 6 contraction chunks over C
NG = S3 // HD       # 36 head-groups per token row
HKC = HID // P      # 24 chunks over HID
NJ = H // 2         # 6 head pairs
W8SCALE = 32.0      # host scale on fp8 mlp weights
ELN16 = -2.772588722239781  # -ln(16): fp8-range shift for exp

_CACHE = {}


def _build(flags):
    import concourse.bass as bass
    import concourse.tile as tile
    from concourse import bacc, mybir

    f32 = mybir.dt.float32
    bf16 = mybir.dt.bfloat16
    f8 = mybir.dt.float8e4
    AF = mybir.ActivationFunctionType
    ALU = mybir.AluOpType
    AX = mybir.AxisListType.X
    DR = mybir.MatmulPerfMode.DoubleRow

    (n1_aff, hln_aff, n2_aff, has_projb, has_fc1b, has_fc2b) = flags

    nc = bacc.Bacc("TRN2", target_bir_lowering=False)

    # ---------------- I/O ----------------
    x_in = {
        "b": nc.dram_tensor("x_b", [N, C], f32, kind="ExternalInput"),
        "a": nc.dram_tensor("x_a", [N, C], f32, kind="ExternalInput"),
    }
    qkv_wT = nc.dram_tensor("qkv_wT", [C, S3], bf16, kind="ExternalInput")
    pw2_d = nc.dram_tensor("pw2", [P, NJ * C], bf16, kind="ExternalInput")
    w1p_d = nc.dram_tensor("w1p", [P, HKC * C], f8, kind="ExternalInput")
    w2p_d = nc.dram_tensor("w2p", [P, HKC * C], f8, kind="ExternalInput")
    projb_d = nc.dram_tensor("proj_b", [1, C], bf16, kind="ExternalInput") if has_projb else None
    fc1b_d = nc.dram_tensor("fc1_b", [HID], f32, kind="ExternalInput") if has_fc1b else None
    fc2b_d = nc.dram_tensor("fc2_b", [1, C], bf16, kind="ExternalInput") if has_fc2b else None
    n1w_d = nc.dram_tensor("norm1_w", [C], f32, kind="ExternalInput") if n1_aff else None
    n1b_d = nc.dram_tensor("norm1_b", [C], f32, kind="ExternalInput") if n1_aff else None
    n2w_d = nc.dram_tensor("norm2_w", [C], f32, kind="ExternalInput") if n2_aff else None
    n2b_d = nc.dram_tensor("norm2_b", [C], f32, kind="ExternalInput") if n2_aff else None
    hlnw_d = nc.dram_tensor("hln_w", [HD], f32, kind="ExternalInput") if hln_aff else None
    hlnb_d = nc.dram_tensor("hln_b", [HD], f32, kind="ExternalInput") if hln_aff else None
    out_d = {
        "b": nc.dram_tensor("out_b", [N, C], f32, kind="ExternalOutput"),
        "a": nc.dram_tensor("out_a", [N, C], f32, kind="ExternalOutput"),
    }

    with tile.TileContext(nc) as tc:
        with (
            tc.tile_pool(name="dram", bufs=1, space="DRAM") as dram,
            tc.tile_pool(name="const", bufs=1) as const,
            tc.tile_pool(name="big", bufs=4) as big,    # xnT/ctx2/hT rotate
            tc.tile_pool(name="s1", bufs=1) as s1,      # weights + va
            tc.tile_pool(name="sB", bufs=2) as sB,
            tc.tile_pool(name="s2", bufs=2) as s2,
            tc.tile_pool(name="sF", bufs=4) as sF,      # f32 chunk buffers
            tc.tile_pool(name="s3", bufs=2) as s3,
            tc.tile_pool(name="s3b", bufs=3) as s3b,
            tc.tile_pool(name="ps", bufs=4, space="PSUM") as ps,
        ):
            # -------- DRAM staging --------
            xn_t = {s: dram.tile([N, C], bf16, name=f"xn_{s}", tag=f"xn_{s}") for s in "ba"}
            qkT_t = {s: dram.tile([2 * C, N], bf16, name=f"qkT_{s}", tag=f"qkT_{s}") for s in "ba"}
            v_t = {s: dram.tile([N, C], f8, name=f"v_{s}", tag=f"v_{s}") for s in "ba"}
            qr_t = {s: dram.tile([H * N, HD], bf16, name=f"qr_{s}", tag=f"qr_{s}") for s in "ba"}
            o_t = {s: dram.tile([N, C], bf16, name=f"o_{s}", tag=f"o_{s}") for s in "ba"}
            x2T_t = {s: dram.tile([C, N], bf16, name=f"x2T_{s}", tag=f"x2T_{s}") for s in "ba"}

            # -------- constants --------
            from concourse.masks import make_identity
            ident = const.tile([P, P], bf16, tag="ident")
            make_identity(nc, ident)
            epsC = const.tile([P, 1], f32, tag="epsC")
            nc.vector.memset(epsC, EPS)
            eln = const.tile([P, 1], f32, tag="eln")
            nc.vector.memset(eln, ELN16)

            if has_projb or has_fc2b:
                ones_bf = const.tile([1, P], bf16, tag="ones_bf")
                nc.vector.memset(ones_bf, 1.0)
            if has_projb:
                projb_sb = const.tile([1, C], bf16, tag="projb")
                nc.sync.dma_start(projb_sb, projb_d[:])
            if has_fc2b:
                fc2b_sb = const.tile([1, C], bf16, tag="fc2b")
                nc.sync.dma_start(fc2b_sb, fc2b_d[:])
            if has_fc1b:
                fc1b_sb = const.tile([P, HKC], f32, tag="fc1b")
                nc.sync.dma_start(fc1b_sb, fc1b_d[:].rearrange("(k p) -> p k", p=P))

            def bcast_load(src_ap, cols, tag):
                t = const.tile([P, cols], f32, tag=tag)
                bc = bass.AP(tensor=src_ap.tensor, offset=src_ap.offset,
                             ap=[[0, P]] + list(src_ap.ap))
                nc.gpsimd.dma_start(out=t, in_=bc)
                return t

            if n1_aff:
                n1w_sb = bcast_load(n1w_d[:], C, "n1w")
                n1b_sb = bcast_load(n1b_d[:], C, "n1b")
            if n2_aff:
                n2w_sb = bcast_load(n2w_d[:], C, "n2w")
                n2b_sb = bcast_load(n2b_d[:], C, "n2b")
            if hln_aff:
                hlnw_sb = bcast_load(hlnw_d[:], HD, "hlnw")
                hlnb_sb = bcast_load(hlnb_d[:], HD, "hlnb")

            # -------- helpers --------
            def ln_stats(x_tile):
                """bn stats over free dim 768 -> (mu [P,1], rstd [P,1])."""
                st = s2.tile([P, 3, 6], f32, tag="lnst", bufs=4)
                for g in range(3):
                    nc.vector.bn_stats(st[:, g, :], x_tile[:, g * 256:(g + 1) * 256])
                mv = s2.tile([P, 2], f32, tag="lnmv", bufs=4)
                nc.vector.bn_aggr(mv, st)
                std = s2.tile([P, 1], f32, tag="lnstd", bufs=4)
                nc.scalar.activation(std, mv[:, 1:2], AF.Sqrt, bias=epsC)
                rstd = s2.tile([P, 1], f32, tag="lnrstd", bufs=4)
                nc.vector.reciprocal(rstd, std)
                return mv, rstd

            def ln_apply(out_tile, x_tile, mv, rstd, w_sb, b_sb):
                nc.vector.tensor_scalar(out_tile, x_tile, mv[:, 0:1], rstd,
                                        ALU.subtract, ALU.mult)
                if w_sb is not None:
                    nc.vector.tensor_tensor(out_tile, out_tile, w_sb, ALU.mult)
                    nc.vector.tensor_tensor(out_tile, out_tile, b_sb, ALU.add)


            # ======== P1 + QKV, streams interleaved per chunk ========
            xnTd = {}
            for s in "ba":
                xnTd[s] = big.tile([P, KC, N], bf16, name=f"xnT_{s}", tag="big")
            wq = s1.tile([P, KC, S3], bf16, tag="wbig")
            nc.scalar.dma_start(wq, qkv_wT[:].rearrange("(k p) f -> p k f", p=P))
            pw2sb = s1.tile([P, NJ, C], bf16, tag="pw2sb")
            nc.scalar.dma_start(pw2sb, pw2_d[:].rearrange("p (j o) -> p j o", o=C))

            xnb_st = {}

            def p1_ln(s, c):
                """LN1 chain (vector) — issued one chunk ahead of p1_tp."""
                cs = slice(c * P, (c + 1) * P)
                with nc.named_scope(f"p1_{s}"):
                    xt = sF.tile([P, C], f32, tag="f32buf", name="xt")
                    nc.sync.dma_start(xt, x_in[s][cs, :])
                    mv, rstd = ln_stats(xt)
                    xnb = s2.tile([P, C], bf16, tag="xnb")
                    ln_apply(xnb, xt, mv, rstd,
                             n1w_sb if n1_aff else None,
                             n1b_sb if n1_aff else None)
                    nc.sync.dma_start(xn_t[s][cs, :], xnb)
                    xnb_st[(s, c)] = xnb

            def p1_tp(s, c):
                cs = slice(c * P, (c + 1) * P)
                xnb = xnb_st.pop((s, c))
                with nc.named_scope(f"p1_{s}"):
                    tp = ps.tile([P, KC, P], bf16, tag="A", name="tp1")
                    for t in range(KC):
                        nc.tensor.transpose(tp[:, t, :], xnb[:, t * P:(t + 1) * P], ident)
                    nc.scalar.copy(xnTd[s][:, :, cs], tp)

            def qkv_chunk2(c):
                """Both streams, section-interleaved: six matmul sections
                (~20us of PE work) cover the per-section stats chains
                (square->reduce->sqrt->recip->normalize) running two-wide
                on vector/scalar, so the q/k transposes never stall PE."""
                cs = slice(c * P, (c + 1) * P)
                zbs = {}
                for s in "ba":
                    with nc.named_scope(f"qkv_{s}"):
                        zbs[s] = s2.tile([P, S3], bf16, tag="zb", name=f"zb_{s}")

                def sect_mm(s, si):
                    f0 = si * C
                    with nc.named_scope(f"qkv_{s}"):
                        acc = ps.tile([P, C], f32, tag="A", name=f"qacc{si}")
                        for k in range(KC):
                            for m0, mw in ((0, 512), (512, 256)):
                                nc.tensor.matmul(
                                    acc[:, m0:m0 + mw],
                                    xnTd[s][:, k, cs],
                                    wq[:, k, f0 + m0:f0 + m0 + mw],
                                    start=(k == 0), stop=(k == KC - 1))
                        return acc

                def sect_norm(s, si, acc):
                    f0 = si * C
                    gw = C // HD  # 12
                    with nc.named_scope(f"qkv_{s}"):
                        sq = s2.tile([P, 1024], bf16, tag="sq")
                        nc.scalar.activation(sq[:, :C], acc, AF.Square)
                        sumsq = s2.tile([P, NG], f32, tag="hsumsq", bufs=3)
                        nc.vector.reduce_sum(
                            sumsq[:, :gw],
                            sq[:, :C].rearrange("p (g d) -> p g d", d=HD),
                            axis=AX)
                        stdq = s2.tile([P, NG], f32, tag="hstd", bufs=3)
                        nc.scalar.activation(stdq[:, :gw], sumsq[:, :gw],
                                             AF.Sqrt, bias=epsC,
                                             scale=1.0 / HD)
                        rstd = s2.tile([P, NG], f32, tag="hrstd", bufs=3)
                        nc.vector.reciprocal(rstd[:, :gw], stdq[:, :gw])
                        zv = zbs[s][:, f0:f0 + C].rearrange(
                            "p (g d) -> p g d", d=HD)
                        nc.vector.tensor_tensor(
                            zv,
                            acc.rearrange("p (g d) -> p g d", d=HD),
                            rstd[:, :gw, None].to_broadcast([P, gw, HD]),
                            ALU.mult)
                        if hln_aff:
                            nc.vector.tensor_tensor(
                                zv, zv,
                                hlnw_sb[:, None, :].to_broadcast([P, gw, HD]),
                                ALU.mult)
                            nc.vector.tensor_tensor(
                                zv, zv,
                                hlnb_sb[:, None, :].to_broadcast([P, gw, HD]),
                                ALU.add)

                def sect_tp(s, half):
                    with nc.named_scope(f"qkv_{s}"):
                        tp2 = ps.tile([P, KC, P], bf16, tag="A", name="tp2")
                        for t in range(KC):
                            tt = half * KC + t
                            nc.tensor.transpose(
                                tp2[:, t, :], zbs[s][:, tt * P:(tt + 1) * P],
                                ident)
                        qkt_sb = s2.tile([P, KC, P], bf16, tag="qkt", bufs=3)
                        nc.scalar.copy(qkt_sb, tp2)
                        nc.sync.dma_start(
                            qkT_t[s][:].rearrange("(t p) n -> p t n", p=P)
                            [:, half * KC:(half + 1) * KC, cs],
                            qkt_sb)

                acc_q = {s: sect_mm(s, 0) for s in "ba"}
                acc_k = {}
                for s in "ba":
                    acc_k[s] = sect_mm(s, 1)
                for s in "ba":
                    sect_norm(s, 0, acc_q[s])
                acc_v = {}
                for s in "ba":
                    acc_v[s] = sect_mm(s, 2)
                for s in "ba":
                    sect_norm(s, 1, acc_k[s])
                for s in "ba":
                    sect_tp(s, 1)            # k transposes first for attn
                for s in "ba":
                    sect_norm(s, 2, acc_v[s])
                for s in "ba":
                    sect_tp(s, 0)
                for s in "ba":
                    with nc.named_scope(f"qkv_{s}"):
                        nc.gpsimd.dma_start(v_t[s][cs, :], zbs[s][:, 2 * C:])
                        nc.sync.dma_start(
                            qr_t[s][:].rearrange("(h n) d -> n h d", h=H)[cs],
                            zbs[s][:, :C].rearrange("p (g d) -> p g d", d=HD))

            for s in "ba":
                p1_ln(s, 0)
            for c in range(NCH):
                for s in "ba":
                    p1_tp(s, c)
                if c + 1 < NCH:
                    for s in "ba":
                        p1_ln(s, c + 1)
                qkv_chunk2(c)

            # ======== attention + proj + mlp, software-pipelined ========
            DIRS = (("b", "a"), ("a", "b"))  # (qs, ks); output goes to stream ks
            ctx2 = {}
            for qs, ks in DIRS:
                ctx2[qs] = big.tile([P, NJ, N], bf16, name=f"ctx2_{qs}", tag="big")
            # persistent [v | ones] stationaries: parity x direction
            vap = {}
            for hp in range(2):
                for qs, ks in DIRS:
                    t = s1.tile([P, NCH, P], f8, tag=f"va{hp}{qs}")
                    nc.gpsimd.memset(t[:, :, (1 - hp) * HD:(2 - hp) * HD], 1.0)
                    vap[(hp, qs)] = t

            # prefetch mlp fc2 weights (shares the wq slot; frees at attn start)
            w2sb = s1.tile([P, HKC, C], f8, tag="wbig")
            nc.gpsimd.dma_start(w2sb, w2p_d[:].rearrange("p (k o) -> p k o", o=C))
            w1v = w1p_d[:].rearrange("p (kc k f) -> p kc k f", k=KC, f=P)

            def attn_head(j, hp):
                """Both directions merged: QK bursts of 8 back-to-back
                matmuls (>3us: PE ramps to max p-state); PV deferred one
                mc2 so it never waits on exp."""
                h = 2 * j + hp
                hs = slice(hp * HD, (hp + 1) * HD)        # ctx half
                ds = slice((1 - hp) * HD, (2 - hp) * HD)  # denominator half
                lo = slice(0, HD)
                qts, kts, cps = {}, {}, {}
                for qs, ks in DIRS:
                    with nc.named_scope(f"attn_{qs}"):
                        qt = s3b.tile([HD, N], bf16, tag="qh", name=f"qh_{qs}{h}")
                        nc.sync.dma_start(qt, qkT_t[qs][h * HD:(h + 1) * HD, :])
                        kt = s3b.tile([HD, N], bf16, tag="kh", name=f"kh_{qs}{h}")
                        nc.sync.dma_start(
                            kt, qkT_t[ks][C + h * HD:C + (h + 1) * HD, :])
                        va = vap[(hp, qs)]
                        nc.sync.dma_start(
                            va[:, :, hp * HD:(hp + 1) * HD],
                            v_t[ks][:].rearrange("(c p) f -> p c f", p=P)
                            [:, :, h * HD:(h + 1) * HD])
                        qts[qs], kts[qs] = qt, kt
                        cps[qs] = ps.tile([P, 2, 512], f32, tag="A",
                                          name=f"cps_{qs}")
                pend = []  # deferred PV: (qs, mc2, pt2)
                for mc2 in range(NCH // 2):
                    cur = []
                    for qs, ks in DIRS:
                        with nc.named_scope(f"attn_{qs}"):
                            # pt2[p, nh, mi, n]: PV k-tile pair contiguous
                            pt2 = s3b.tile([P, 2, 2, 512], f8, tag="pt",
                                           bufs=4)
                            for mi in range(2):
                                mc = 2 * mc2 + mi
                                sps = ps.tile([P, 2, 512], f32, tag="A",
                                              name="sps")
                                for nh in range(2):
                                    nc.tensor.matmul(
                                        sps[:, nh, :],
                                        kts[qs][:, mc * P:(mc + 1) * P],
                                        qts[qs][:, nh * 512:(nh + 1) * 512])
                                nc.scalar.activation(
                                    pt2[:, :, mi, :], sps,
                                    AF.Exp, scale=float(HD ** -0.5), bias=eln)
                            cur.append((qs, mc2, pt2))
                    for qs, pmc2, pt2 in pend:
                        with nc.named_scope(f"attn_{qs}"):
                            for nh in range(2):
                                nc.tensor.matmul(
                                    cps[qs][:, nh, :],
                                    vap[(hp, qs)][:, 2 * pmc2:2 * pmc2 + 2, :],
                                    pt2[:, nh],
                                    perf_mode=DR,
                                    start=(pmc2 == 0),
                                    stop=(pmc2 == NCH // 2 - 1))
                    pend = cur
                for qs, pmc2, pt2 in pend:
                    with nc.named_scope(f"attn_{qs}"):
                        for nh in range(2):
                            nc.tensor.matmul(
                                cps[qs][:, nh, :],
                                vap[(hp, qs)][:, 2 * pmc2:2 * pmc2 + 2, :],
                                pt2[:, nh],
                                perf_mode=DR,
                                start=(pmc2 == 0),
                                stop=(pmc2 == NCH // 2 - 1))
                for qs, ks in DIRS:
                    with nc.named_scope(f"attn_{qs}"):
                        # denominator (replicated on partitions ds):
                        # aligned copy out of PSUM, shift to base 0,
                        # recipfast at base 0, shift to hs, aligned mult.
                        dn = s3.tile([P, N], f32, tag="dn")
                        nc.vector.tensor_copy(
                            dn[ds, :],
                            cps[qs][ds, :, :].rearrange("p a b -> p (a b)"))
                        if hp == 0:
                            nc.gpsimd.dma_start(dn[lo, :], dn[ds, :])
                        rd = s3.tile([P, N], f32, tag="rd")
                        nc.vector.reciprocal_approx_fast(rd[lo, :], dn[lo, :])
                        if hp == 1:
                            nc.gpsimd.dma_start(rd[hs, :], rd[lo, :])
                        nc.vector.tensor_tensor(
                            ctx2[qs][hs, j, :],
                            cps[qs][hs, :, :].rearrange("p a b -> p (a b)"),
                            rd[hs, :], ALU.mult)

            def proj_mm(qs, ks, c):
                """proj matmul + residual + LN2 stats for chunk c."""
                cs = slice(c * P, (c + 1) * P)
                with nc.named_scope(f"proj_{ks}"):
                    qr_view = qr_t[qs][:].rearrange("(n j) d -> n (j d)", j=H)
                    y = ps.tile([P, C], f32, tag="A", name="yproj")
                    for jj in range(NJ):
                        for o0, ow in ((0, 512), (512, 256)):
                            nc.tensor.matmul(
                                y[:, o0:o0 + ow],
                                ctx2[qs][:, jj, cs],
                                pw2sb[:, jj, o0:o0 + ow],
                                start=(jj == 0),
                                stop=(jj == NJ - 1 and not has_projb))
                    if has_projb:
                        for o0, ow in ((0, 512), (512, 256)):
                            nc.tensor.matmul(
                                y[:, o0:o0 + ow], ones_bf[0:1, :],
                                projb_sb[0:1, o0:o0 + ow],
                                start=False, stop=True)
                    xnr = s2.tile([P, C], bf16, tag="xnr", bufs=3)
                    nc.sync.dma_start(xnr, xn_t[ks][cs, :])
                    qres = s2.tile([P, C], bf16, tag="qres", bufs=3)
                    nc.sync.dma_start(qres, qr_view[cs, :])
                    t1 = sF.tile([P, C], f32, tag="t1f", name="t1", bufs=3)
                    nc.vector.tensor_tensor(t1, y, xnr, ALU.add)
                    ot = s2.tile([P, C], bf16, tag="otb", bufs=3)
                    nc.vector.tensor_tensor(ot, t1, qres, ALU.add)
                    nc.sync.dma_start(o_t[ks][cs, :], ot)
                    mv, rstd = ln_stats(ot)
                    return (ks, c, ot, mv, rstd)

            def proj_tail(state):
                """LN2 apply + transpose + store, one chunk behind proj_mm."""
                ks, c, ot, mv, rstd = state
                cs = slice(c * P, (c + 1) * P)
                with nc.named_scope(f"proj_{ks}"):
                    x2b = s2.tile([P, C], bf16, tag="x2b", bufs=3)
                    ln_apply(x2b, ot, mv, rstd,
                             n2w_sb if n2_aff else None,
                             n2b_sb if n2_aff else None)
                    tp3 = ps.tile([P, KC, P], bf16, tag="A", name="tp3")
                    for t in range(KC):
                        nc.tensor.transpose(tp3[:, t, :], x2b[:, t * P:(t + 1) * P],
                                            ident)
                    x2ts = s2.tile([P, KC, P], bf16, tag="x2ts", bufs=3)
                    nc.scalar.copy(x2ts, tp3)
                    nc.sync.dma_start(
                        x2T_t[ks][:].rearrange("(t p) n -> p t n", p=P)[:, :, cs],
                        x2ts)

            # MLP state per stream
            mlp_x2h = {}
            mlp_hT = {}

            def mlp_load(s):
                with nc.named_scope(f"mlp_{s}"):
                    x2h = []
                    for nh in range(2):
                        xh = sB.tile([P, KC, 512], f8, tag="x2h", bufs=4,
                                     name=f"x2h_{s}{nh}")
                        nc.gpsimd.dma_start(
                            out=xh,
                            in_=x2T_t[s][:].rearrange("(k p) n -> p k n", p=P)
                            [:, :, nh * 512:(nh + 1) * 512])
                        x2h.append(xh)
                    mlp_x2h[s] = x2h
                    hT = []
                    for nh in range(2):
                        hT.append(big.tile([P, HKC, 512], f8, tag="big",
                                           name=f"hT_{s}{nh}"))
                    mlp_hT[s] = hT

            def fc1_step(kc2, streams="ab"):
                """fc1 at kc2 for given streams."""
                w1k = []
                for kk in range(2):
                    w1t = s3b.tile([P, KC, P], f8, tag="w1k", bufs=3)
                    nc.sync.dma_start(w1t, w1v[:, 2 * kc2 + kk, :, :])
                    w1k.append(w1t)
                for s in streams:
                    x2h = mlp_x2h[s]
                    hT = mlp_hT[s]
                    with nc.named_scope(f"mlp_{s}"):
                        # kk-major: gelu(kk=0) overlaps the kk=1 matmuls
                        accs = [ps.tile([P, 2, 512], f32, tag="A",
                                        name=f"facc{nh}")
                                for nh in range(2)]
                        for kk in range(2):
                            for kp in range(0, KC, 2):
                                for nh in range(2):
                                    nc.tensor.matmul(
                                        accs[nh][:, kk, :],
                                        w1k[kk][:, kp:kp + 2, :],
                                        x2h[nh][:, kp:kp + 2, :],
                                        perf_mode=DR,
                                        start=(kp == 0), stop=(kp == KC - 2))
                            kc = 2 * kc2 + kk
                            for nh in range(2):
                                if has_fc1b:
                                    nc.scalar.activation(
                                        hT[nh][:, kc, :], accs[nh][:, kk, :],
                                        AF.Gelu, bias=fc1b_sb[:, kc:kc + 1],
                                        scale=1.0 / W8SCALE)
                                else:
                                    nc.scalar.activation(
                                        hT[nh][:, kc, :], accs[nh][:, kk, :],
                                        AF.Gelu, scale=1.0 / W8SCALE)

            def fc2_chunk(s, nh, sub):
                hT = mlp_hT[s]
                c = nh * 4 + sub
                cs = slice(c * P, (c + 1) * P)
                with nc.named_scope(f"mlp_{s}"):
                    y = ps.tile([P, C], f32, tag="A", name="yfc2")
                    for kc in range(0, HKC, 2):
                        for o0, ow in ((0, 512), (512, 256)):
                            nc.tensor.matmul(
                                y[:, o0:o0 + ow],
                                hT[nh][:, kc:kc + 2, sub * P:(sub + 1) * P],
                                w2sb[:, kc:kc + 2, o0:o0 + ow],
                                perf_mode=DR,
                                start=(kc == 0),
                                stop=(kc == HKC - 2 and not has_fc2b))
                    if has_fc2b:
                        for o0, ow in ((0, 512), (512, 256)):
                            nc.tensor.matmul(
                                y[:, o0:o0 + ow], ones_bf[0:1, :],
                                fc2b_sb[0:1, o0:o0 + ow],
                                start=False, stop=True)
                    oh = s2.tile([P, C], bf16, tag="oh")
                    nc.sync.dma_start(oh, o_t[s][cs, :])
                    outt = sF.tile([P, C], f32, tag="f32buf", name="outt")
                    nc.vector.scalar_tensor_tensor(
                        outt, y, 1.0 / W8SCALE, oh, ALU.mult, ALU.add)
                    nc.sync.dma_start(out_d[s][cs, :], outt)

            # ---- attention: both dirs merged per head ----
            for j in range(NJ):
                for hp in range(2):
                    attn_head(j, hp)

            # ---- proj both streams, alternating chunks, tail skewed ----
            prev = []
            for c in range(NCH):
                cur = [proj_mm("b", "a", c), proj_mm("a", "b", c)]
                for st in prev:
                    proj_tail(st)
                prev = cur
            for st in prev:
                proj_tail(st)
            mlp_load("a")
            mlp_load("b")
            for kc2 in range(HKC // 2):
                fc1_step(kc2, "a")
            FC2A = {0: [0], 1: [1], 2: [2], 3: [3], 4: [4], 5: [5],
                    6: [6], 7: [7], 8: [], 9: [], 10: [], 11: []}
            for kc2 in range(HKC // 2):
                fc1_step(kc2, "b")
                for idx in FC2A.get(kc2, []):
                    fc2_chunk("a", idx // 4, idx % 4)
            for idx in range(8):
                fc2_chunk("b", idx // 4, idx % 4)

    nc.finalize()
    return nc


def _get_nc(flags):
    if flags not in _CACHE:
        _CACHE[flags] = _build(flags)
    return _CACHE[flags]


def _prep(inputs):
    import ml_dtypes

    f = np.float32
    bf = ml_dtypes.bfloat16
    f8 = ml_dtypes.float8_e4m3
    w = {k: np.asarray(v, f) for k, v in inputs.items()}
    flags = (
        not (np.all(w["norm1_w"] == 1) and np.all(w["norm1_b"] == 0)),
        not (np.all(w["hln_w"] == 1) and np.all(w["hln_b"] == 0)),
        not (np.all(w["norm2_w"] == 1) and np.all(w["norm2_b"] == 0)),
        bool(np.any(w["proj_b"] != 0)),
        bool(np.any(w["fc1_b"] != 0)),
        bool(np.any(w["fc2_b"] != 0)),
    )
    # qkv weights: transpose + fold head-LN centering (linear in x)
    wT = np.ascontiguousarray(w["qkv_w"].T)                   # [C, 3C]
    wT3 = wT.reshape(C, NG, HD)
    wTc = (wT3 - wT3.mean(axis=2, keepdims=True)).reshape(C, S3)
    # proj weights packed by head pair: pw2[p=(h%2)*64+d, j=h//2, o]
    pw = w["proj_w"].T.reshape(NJ, 2, HD, C).transpose(1, 2, 0, 3).reshape(P, NJ * C)
    # fc1 packed: w1p[p, kc, k, f'] = 32*fc1_w[kc*128+f', k*128+p], fp8
    w1p = (W8SCALE * w["fc1_w"]).reshape(HKC, P, KC, P).transpose(3, 0, 2, 1).reshape(P, HKC * C)
    # fc2 packed: w2p[p, kc, o] = 32*fc2_w[o, kc*128+p], fp8
    w2p = (W8SCALE * w["fc2_w"]).reshape(C, HKC, P).transpose(2, 1, 0).reshape(P, HKC * C)
    shared = {
        "qkv_wT": wTc.astype(bf),
        "pw2": np.ascontiguousarray(pw).astype(bf),
        "w1p": np.ascontiguousarray(w1p).astype(f8),
        "w2p": np.ascontiguousarray(w2p).astype(f8),
    }
    n1_aff, hln_aff, n2_aff, pb, f1b, f2b = flags
    if pb:
        shared["proj_b"] = w["proj_b"].reshape(1, C).astype(bf)
    if f1b:
        shared["fc1_b"] = w["fc1_b"]
    if f2b:
        shared["fc2_b"] = (w["fc2_b"] * W8SCALE).reshape(1, C).astype(bf)
    if n1_aff:
        shared["norm1_w"] = w["norm1_w"]
        shared["norm1_b"] = w["norm1_b"]
    if n2_aff:
        shared["norm2_w"] = w["norm2_w"]
        shared["norm2_b"] = w["norm2_b"]
    if hln_aff:
        shared["hln_w"] = w["hln_w"]
        shared["hln_b"] = w["hln_b"]
    return w, flags, shared


def kernel(trace=False, **inputs):
    from concourse.bass_utils import run_bass_kernel_spmd

    w, flags, shared = _prep(inputs)
    nc = _get_nc(flags)
    before = np.ascontiguousarray(w["before"], dtype=np.float32)
    after = np.ascontiguousarray(w["after"], dtype=np.float32)
    in_maps = []
    for core in range(B):
        m = dict(shared)
        m["x_b"] = np.ascontiguousarray(before[core])
        m["x_a"] = np.ascontiguousarray(after[core])
        in_maps.append(m)
    res = run_bass_kernel_spmd(nc, in_maps, core_ids=list(range(B)), trace=trace)
    before_o = np.stack([res.results[i]["out_b"] for i in range(B)])
    after_o = np.stack([res.results[i]["out_a"] for i in range(B)])
    out = (before_o.astype(np.float32), after_o.astype(np.float32))
    if trace:
        return out, res
    return out


# revision 60
# speedup vs baseline: 1.0224x; 1.0224x over previous
"""Trainium2 Bass kernel for nn_Block_22325240004804 (dense_transformer).

Two-stream cross-attention transformer block, B=8 N=1024 C=768 H=12.
Sharding: pure data parallel - batch element b on core b (no collectives).

v8 (on top of v5; 901us -> ~670us):
  - merged-direction attention: per (head, parity) both directions'
    QK matmuls issue as one 8-instruction burst, exps for dir1 overlap
    dir2's QKs, and PV is deferred one mc2 so it never waits on exp
    (PE p-state: only >3us continuous bursts reach max clock; the old
    per-dir chains ran everything at mid/low clock)
  - pt2 layout [P, nh, mi, n]: PV DoubleRow k-tile pair contiguous
  - phase1: LN1 chain issued one chunk ahead of its transposes; QKV
    sections aligned to q/k/v (768 each) and BOTH streams' sections
    interleaved, so ~20us of matmuls cover the per-section stats chains
    (square->reduce->sqrt->recip->normalize) running two-wide on
    vector/scalar; k-transposes stored before q so attention's first
    loads unblock sooner
  - bf16 staging for xn/o residuals (halves staging DMA, kills casts);
    xnT copies on scalar; LN2 + residual adds on vector (scalar is
    exp/gelu-critical); denominator shift DMAs on gpsimd
  - proj software-pipelined: LN2-apply/transpose stage skewed one chunk
    behind matmul/residual; mlp: fc1_a, then fc1_b || fc2_a, then fc2_b
    with kk-major fc1 so gelu(kk=0) overlaps kk=1 matmuls
  - w2 prefetched at attention start (reuses wq's SBUF slot)
  Rejected experimentally: fp8 DoubleRow QK (no DR speedup at mid
  p-state, +LDW cost), fp8 QKV (weight quantization error 2.7e-2 >
  2e-2 gate), gpsimd tensor ops on critical paths (too slow).
"""

import sys

if "/opt/trn_rl_repo" not in sys.path:
    sys.path.insert(0, "/opt/trn_rl_repo")

import numpy as np

B, N, C = 8, 1024, 768
H, HD = 12, 64
S3 = 3 * C          # 2304
HID = 4 * C         # 3072
EPS = 1e-5
P = 128
NCH = N // P        # 8 token chunks
KC = C // P         # 6 contraction chunks over C
NG = S3 // HD       # 36 head-groups per token row
HKC = HID // P      # 24 chunks over HID
NJ = H // 2         # 6 head pairs
W8SCALE = 32.0      # host scale on fp8 mlp weights
ELN16 = -2.772588722239781  # -ln(16): fp8-range shift for exp

_CACHE = {}


def _build(flags):
    import concourse.bass as bass
    import concourse.tile as tile
    from concourse import bacc, mybir

    f32 = mybir.dt.float32
    bf16 = mybir.dt.bfloat16
    f8 = mybir.dt.float8e4
    AF = mybir.ActivationFunctionType
    ALU = mybir.AluOpType
    AX = mybir.AxisListType.X
    DR = mybir.MatmulPerfMode.DoubleRow

    (n1_aff, hln_aff, n2_aff, has_projb, has_fc1b, has_fc2b) = flags

    nc = bacc.Bacc("TRN2", target_bir_lowering=False)

    # ---------------- I/O ----------------
    x_in = {
        "b": nc.dram_tensor("x_b", [N, C], f32, kind="ExternalInput"),
        "a": nc.dram_tensor("x_a", [N, C], f32, kind="ExternalInput"),
    }
    qkv_wT = nc.dram_tensor("qkv_wT", [C, S3], bf16, kind="ExternalInput")
    pw2_d = nc.dram_tensor("pw2", [P, NJ * C], bf16, kind="ExternalInput")
    w1p_d = nc.dram_tensor("w1p", [P, HKC * C], f8, kind="ExternalInput")
    w2p_d = nc.dram_tensor("w2p", [P, HKC * C], f8, kind="ExternalInput")
    projb_d = nc.dram_tensor("proj_b", [1, C], bf16, kind="ExternalInput") if has_projb else None
    fc1b_d = nc.dram_tensor("fc1_b", [HID], f32, kind="ExternalInput") if has_fc1b else None
    fc2b_d = nc.dram_tensor("fc2_b", [1, C], bf16, kind="ExternalInput") if has_fc2b else None
    n1w_d = nc.dram_tensor("norm1_w", [C], f32, kind="ExternalInput") if n1_aff else None
    n1b_d = nc.dram_tensor("norm1_b", [C], f32, kind="ExternalInput") if n1_aff else None
    n2w_d = nc.dram_tensor("norm2_w", [C], f32, kind="ExternalInput") if n2_aff else None
    n2b_d = nc.dram_tensor("norm2_b", [C], f32, kind="ExternalInput") if n2_aff else None
    hlnw_d = nc.dram_tensor("hln_w", [HD], f32, kind="ExternalInput") if hln_aff else None
    hlnb_d = nc.dram_tensor("hln_b", [HD], f32, kind="ExternalInput") if hln_aff else None
    out_d = {
        "b": nc.dram_tensor("out_b", [N, C], f32, kind="ExternalOutput"),
        "a": nc.dram_tensor("out_a", [N, C], f32, kind="ExternalOutput"),
    }

    with tile.TileContext(nc) as tc:
        with (
            tc.tile_pool(name="dram", bufs=1, space="DRAM") as dram,
            tc.tile_pool(name="const", bufs=1) as const,
            tc.tile_pool(name="big", bufs=4) as big,    # xnT/ctx2/hT rotate
            tc.tile_pool(name="s1", bufs=1) as s1,      # weights + va
            tc.tile_pool(name="sB", bufs=2) as sB,
            tc.tile_pool(name="s2", bufs=2) as s2,
            tc.tile_pool(name="sF", bufs=4) as sF,      # f32 chunk buffers
            tc.tile_pool(name="s3", bufs=2) as s3,
            tc.tile_pool(name="s3b", bufs=3) as s3b,
            tc.tile_pool(name="ps", bufs=4, space="PSUM") as ps,
        ):
            # -------- DRAM staging --------
            xn_t = {s: dram.tile([N, C], bf16, name=f"xn_{s}", tag=f"xn_{s}") for s in "ba"}
            qkT_t = {s: dram.tile([2 * C, N], bf16, name=f"qkT_{s}", tag=f"qkT_{s}") for s in "ba"}
            v_t = {s: dram.tile([N, C], f8, name=f"v_{s}", tag=f"v_{s}") for s in "ba"}
            qr_t = {s: dram.tile([H * N, HD], bf16, name=f"qr_{s}", tag=f"qr_{s}") for s in "ba"}
            o_t = {s: dram.tile([N, C], bf16, name=f"o_{s}", tag=f"o_{s}") for s in "ba"}
            x2T_t = {s: dram.tile([C, N], bf16, name=f"x2T_{s}", tag=f"x2T_{s}") for s in "ba"}

            # -------- constants --------
            from concourse.masks import make_identity
            ident = const.tile([P, P], bf16, tag="ident")
            make_identity(nc, ident)
            epsC = const.tile([P, 1], f32, tag="epsC")
            nc.vector.memset(epsC, EPS)
            eln = const.tile([P, 1], f32, tag="eln")
            nc.vector.memset(eln, ELN16)

            if has_projb or has_fc2b:
                ones_bf = const.tile([1, P], bf16, tag="ones_bf")
                nc.vector.memset(ones_bf, 1.0)
            if has_projb:
                projb_sb = const.tile([1, C], bf16, tag="projb")
                nc.sync.dma_start(projb_sb, projb_d[:])
            if has_fc2b:
                fc2b_sb = const.tile([1, C], bf16, tag="fc2b")
                nc.sync.dma_start(fc2b_sb, fc2b_d[:])
            if has_fc1b:
                fc1b_sb = const.tile([P, HKC], f32, tag="fc1b")
                nc.sync.dma_start(fc1b_sb, fc1b_d[:].rearrange("(k p) -> p k", p=P))

            def bcast_load(src_ap, cols, tag):
                t = const.tile([P, cols], f32, tag=tag)
                bc = bass.AP(tensor=src_ap.tensor, offset=src_ap.offset,
                             ap=[[0, P]] + list(src_ap.ap))
                nc.gpsimd.dma_start(out=t, in_=bc)
                return t

            if n1_aff:
                n1w_sb = bcast_load(n1w_d[:], C, "n1w")
                n1b_sb = bcast_load(n1b_d[:], C, "n1b")
            if n2_aff:
                n2w_sb = bcast_load(n2w_d[:], C, "n2w")
                n2b_sb = bcast_load(n2b_d[:], C, "n2b")
            if hln_aff:
                hlnw_sb = bcast_load(hlnw_d[:], HD, "hlnw")
                hlnb_sb = bcast_load(hlnb_d[:], HD, "hlnb")

            # -------- helpers --------
            def ln_stats(x_tile):
                """bn stats over free dim 768 -> (mu [P,1], rstd [P,1])."""
                st = s2.tile([P, 3, 6], f32, tag="lnst", bufs=4)
                for g in range(3):
                    nc.vector.bn_stats(st[:, g, :], x_tile[:, g * 256:(g + 1) * 256])
                mv = s2.tile([P, 2], f32, tag="lnmv", bufs=4)
                nc.vector.bn_aggr(mv, st)
                std = s2.tile([P, 1], f32, tag="lnstd", bufs=4)
                nc.scalar.activation(std, mv[:, 1:2], AF.Sqrt, bias=epsC)
                rstd = s2.tile([P, 1], f32, tag="lnrstd", bufs=4)
                nc.vector.reciprocal(rstd, std)
                return mv, rstd

            def ln_apply(out_tile, x_tile, mv, rstd, w_sb, b_sb):
                nc.vector.tensor_scalar(out_tile, x_tile, mv[:, 0:1], rstd,
                                        ALU.subtract, ALU.mult)
                if w_sb is not None:
                    nc.vector.tensor_tensor(out_tile, out_tile, w_sb, ALU.mult)
                    nc.vector.tensor_tensor(out_tile, out_tile, b_sb, ALU.add)


            # ======== P1 + QKV, streams interleaved per chunk ========
            xnTd = {}
            for s in "ba":
                xnTd[s] = big.tile([P, KC, N], bf16, name=f"xnT_{s}", tag="big")
            wq = s1.tile([P, KC, S3], bf16, tag="wbig")
            nc.scalar.dma_start(wq, qkv_wT[:].rearrange("(k p) f -> p k f", p=P))
            pw2sb = s1.tile([P, NJ, C], bf16, tag="pw2sb")
            nc.scalar.dma_start(pw2sb, pw2_d[:].rearrange("p (j o) -> p j o", o=C))

            xnb_st = {}

            def p1_ln(s, c):
                """LN1 chain (vector) — issued one chunk ahead of p1_tp."""
                cs = slice(c * P, (c + 1) * P)
                with nc.named_scope(f"p1_{s}"):
                    xt = sF.tile([P, C], f32, tag="f32buf", name="xt")
                    nc.sync.dma_start(xt, x_in[s][cs, :])
                    mv, rstd = ln_stats(xt)
                    xnb = s2.tile([P, C], bf16, tag="xnb")
                    ln_apply(xnb, xt, mv, rstd,
                             n1w_sb if n1_aff else None,
                             n1b_sb if n1_aff else None)
                    nc.sync.dma_start(xn_t[s][cs, :], xnb)
                    xnb_st[(s, c)] = xnb

            def p1_tp(s, c):
                cs = slice(c * P, (c + 1) * P)
                xnb = xnb_st.pop((s, c))
                with nc.named_scope(f"p1_{s}"):
                    tp = ps.tile([P, KC, P], bf16, tag="A", name="tp1")
                    for t in range(KC):
                        nc.tensor.transpose(tp[:, t, :], xnb[:, t * P:(t + 1) * P], ident)
                    nc.scalar.copy(xnTd[s][:, :, cs], tp)

            def qkv_chunk2(c):
                """Both streams, section-interleaved: six matmul sections
                (~20us of PE work) cover the per-section stats chains
                (square->reduce->sqrt->recip->normalize) running two-wide
                on vector/scalar, so the q/k transposes never stall PE."""
                cs = slice(c * P, (c + 1) * P)
                zbs = {}
                for s in "ba":
                    with nc.named_scope(f"qkv_{s}"):
                        zbs[s] = s2.tile([P, S3], bf16, tag="zb", name=f"zb_{s}")

                def sect_mm(s, si):
                    f0 = si * C
                    with nc.named_scope(f"qkv_{s}"):
                        acc = ps.tile([P, C], f32, tag="A", name=f"qacc{si}")
                        for k in range(KC):
                            for m0, mw in ((0, 512), (512, 256)):
                                nc.tensor.matmul(
                                    acc[:, m0:m0 + mw],
                                    xnTd[s][:, k, cs],
                                    wq[:, k, f0 + m0:f0 + m0 + mw],
                                    start=(k == 0), stop=(k == KC - 1))
                        return acc

                def sect_norm(s, si, acc):
                    f0 = si * C
                    gw = C // HD  # 12
                    with nc.named_scope(f"qkv_{s}"):
                        sq = s2.tile([P, 1024], bf16, tag="sq")
                        nc.scalar.activation(sq[:, :C], acc, AF.Square)
                        sumsq = s2.tile([P, NG], f32, tag="hsumsq", bufs=3)
                        nc.vector.reduce_sum(
                            sumsq[:, :gw],
                            sq[:, :C].rearrange("p (g d) -> p g d", d=HD),
                            axis=AX)
                        stdq = s2.tile([P, NG], f32, tag="hstd", bufs=3)
                        nc.scalar.activation(stdq[:, :gw], sumsq[:, :gw],
                                             AF.Sqrt, bias=epsC,
                                             scale=1.0 / HD)
                        rstd = s2.tile([P, NG], f32, tag="hrstd", bufs=3)
                        nc.vector.reciprocal(rstd[:, :gw], stdq[:, :gw])
                        zv = zbs[s][:, f0:f0 + C].rearrange(
                            "p (g d) -> p g d", d=HD)
                        nc.vector.tensor_tensor(
                            zv,
                            acc.rearrange("p (g d) -> p g d", d=HD),
                            rstd[:, :gw, None].to_broadcast([P, gw, HD]),
                            ALU.mult)
                        if hln_aff:
                            nc.vector.tensor_tensor(
                                zv, zv,
                                hlnw_sb[:, None, :].to_broadcast([P, gw, HD]),
                                ALU.mult)
                            nc.vector.tensor_tensor(
                                zv, zv,
                                hlnb_sb[:, None, :].to_broadcast([P, gw, HD]),
                                ALU.add)

                def sect_tp(s, half):
                    with nc.named_scope(f"qkv_{s}"):
                        tp2 = ps.tile([P, KC, P], bf16, tag="A", name="tp2")
                        for t in range(KC):
                            tt = half * KC + t
                            nc.tensor.transpose(
                                tp2[:, t, :], zbs[s][:, tt * P:(tt + 1) * P],
                                ident)
                        qkt_sb = s2.tile([P, KC, P], bf16, tag="qkt", bufs=3)
                        nc.scalar.copy(qkt_sb, tp2)
                        nc.sync.dma_start(
                            qkT_t[s][:].rearrange("(t p) n -> p t n", p=P)
                            [:, half * KC:(half + 1) * KC, cs],
                            qkt_sb)

                acc_q = {s: sect_mm(s, 0) for s in "ba"}
                acc_k = {}
                for s in "ba":
                    acc_k[s] = sect_mm(s, 1)
                for s in "ba":
                    sect_norm(s, 0, acc_q[s])
                acc_v = {}
                for s in "ba":
                    acc_v[s] = sect_mm(s, 2)
                for s in "ba":
                    sect_norm(s, 1, acc_k[s])
                for s in "ba":
                    sect_tp(s, 1)            # k transposes first for attn
                for s in "ba":
                    sect_norm(s, 2, acc_v[s])
                for s in "ba":
                    sect_tp(s, 0)
                for s in "ba":
                    with nc.named_scope(f"qkv_{s}"):
                        nc.gpsimd.dma_start(v_t[s][cs, :], zbs[s][:, 2 * C:])
                        nc.sync.dma_start(
                            qr_t[s][:].rearrange("(h n) d -> n h d", h=H)[cs],
                            zbs[s][:, :C].rearrange("p (g d) -> p g d", d=HD))

            for s in "ba":
                p1_ln(s, 0)
            for c in range(NCH):
                for s in "ba":
                    p1_tp(s, c)
                if c + 1 < NCH:
                    for s in "ba":
                        p1_ln(s, c + 1)
                qkv_chunk2(c)

            # ======== attention + proj + mlp, software-pipelined ========
            DIRS = (("b", "a"), ("a", "b"))  # (qs, ks); output goes to stream ks
            ctx2 = {}
            for qs, ks in DIRS:
                ctx2[qs] = big.tile([P, NJ, N], bf16, name=f"ctx2_{qs}", tag="big")
            # persistent [v | ones] stationaries: parity x direction
            vap = {}
            for hp in range(2):
                for qs, ks in DIRS:
                    t = s1.tile([P, NCH, P], f8, tag=f"va{hp}{qs}")
                    nc.gpsimd.memset(t[:, :, (1 - hp) * HD:(2 - hp) * HD], 1.0)
                    vap[(hp, qs)] = t

            # prefetch mlp fc2 weights (shares the wq slot; frees at attn start)
            w2sb = s1.tile([P, HKC, C], f8, tag="wbig")
            nc.gpsimd.dma_start(w2sb, w2p_d[:].rearrange("p (k o) -> p k o", o=C))
            w1v = w1p_d[:].rearrange("p (kc k f) -> p kc k f", k=KC, f=P)

            def attn_head(j, hp):
                """Both directions merged: QK bursts of 8 back-to-back
                matmuls (>3us: PE ramps to max p-state); PV deferred one
                mc2 so it never waits on exp."""
                h = 2 * j + hp
                hs = slice(hp * HD, (hp + 1) * HD)        # ctx half
                ds = slice((1 - hp) * HD, (2 - hp) * HD)  # denominator half
                lo = slice(0, HD)
                qts, kts, cps = {}, {}, {}
                for qs, ks in DIRS:
                    with nc.named_scope(f"attn_{qs}"):
                        qt = s3b.tile([HD, N], bf16, tag="qh", name=f"qh_{qs}{h}")
                        nc.sync.dma_start(qt, qkT_t[qs][h * HD:(h + 1) * HD, :])
                        kt = s3b.tile([HD, N], bf16, tag="kh", name=f"kh_{qs}{h}")
                        nc.sync.dma_start(
                            kt, qkT_t[ks][C + h * HD:C + (h + 1) * HD, :])
                        va = vap[(hp, qs)]
                        nc.sync.dma_start(
                            va[:, :, hp * HD:(hp + 1) * HD],
                            v_t[ks][:].rearrange("(c p) f -> p c f", p=P)
                            [:, :, h * HD:(h + 1) * HD])
                        qts[qs], kts[qs] = qt, kt
                        cps[qs] = ps.tile([P, 2, 512], f32, tag="A",
                                          name=f"cps_{qs}")
                pend = []  # deferred PV: (qs, mc2, pt2)
                for mc2 in range(NCH // 2):
                    cur = []
                    for qs, ks in DIRS:
                        with nc.named_scope(f"attn_{qs}"):
                            # pt2[p, nh, mi, n]: PV k-tile pair contiguous
                            pt2 = s3b.tile([P, 2, 2, 512], f8, tag="pt",
                                           bufs=4)
                            for mi in range(2):
                                mc = 2 * mc2 + mi
                                sps = ps.tile([P, 2, 512], f32, tag="A",
                                              name="sps")
                                for nh in range(2):
                                    nc.tensor.matmul(
                                        sps[:, nh, :],
                                        kts[qs][:, mc * P:(mc + 1) * P],
                                        qts[qs][:, nh * 512:(nh + 1) * 512])
                                nc.scalar.activation(
                                    pt2[:, :, mi, :], sps,
                                    AF.Exp, scale=float(HD ** -0.5), bias=eln)
                            cur.append((qs, mc2, pt2))
                    for qs, pmc2, pt2 in pend:
                        with nc.named_scope(f"attn_{qs}"):
                            for nh in range(2):
                                nc.tensor.matmul(
                                    cps[qs][:, nh, :],
                                    vap[(hp, qs)][:, 2 * pmc2:2 * pmc2 + 2, :],
                                    pt2[:, nh],
                                    perf_mode=DR,
                                    start=(pmc2 == 0),
                                    stop=(pmc2 == NCH // 2 - 1))
                    pend = cur
                for qs, pmc2, pt2 in pend:
                    with nc.named_scope(f"attn_{qs}"):
                        for nh in range(2):
                            nc.tensor.matmul(
                                cps[qs][:, nh, :],
                                vap[(hp, qs)][:, 2 * pmc2:2 * pmc2 + 2, :],
                                pt2[:, nh],
                                perf_mode=DR,
                                start=(pmc2 == 0),
                                stop=(pmc2 == NCH // 2 - 1))
                for qs, ks in DIRS:
                    with nc.named_scope(f"attn_{qs}"):
                        # denominator (replicated on partitions ds):
                        # aligned copy out of PSUM, shift to base 0,
                        # recipfast at base 0, shift to hs, aligned mult.
                        dn = s3.tile([P, N], f32, tag="dn")
                        nc.vector.tensor_copy(
                            dn[ds, :],
                            cps[qs][ds, :, :].rearrange("p a b -> p (a b)"))
                        if hp == 0:
                            nc.gpsimd.dma_start(dn[lo, :], dn[ds, :])
                        rd = s3.tile([P, N], f32, tag="rd")
                        nc.vector.reciprocal_approx_fast(rd[lo, :], dn[lo, :])
                        if hp == 1:
                            nc.gpsimd.dma_start(rd[hs, :], rd[lo, :])
                        nc.vector.tensor_tensor(
                            ctx2[qs][hs, j, :],
                            cps[qs][hs, :, :].rearrange("p a b -> p (a b)"),
                            rd[hs, :], ALU.mult)

            def proj_mm(qs, ks, c):
                """proj matmul + residual + LN2 stats for chunk c."""
                cs = slice(c * P, (c + 1) * P)
                with nc.named_scope(f"proj_{ks}"):
                    qr_view = qr_t[qs][:].rearrange("(n j) d -> n (j d)", j=H)
                    xnr = s2.tile([P, C], bf16, tag="xnr", bufs=3)
                    nc.sync.dma_start(xnr, xn_t[ks][cs, :])
                    qres = s2.tile([P, C], bf16, tag="qres", bufs=3)
                    nc.sync.dma_start(qres, qr_view[cs, :])
                    y = ps.tile([P, C], f32, tag="A", name="yproj")
                    for jj in range(NJ):
                        for o0, ow in ((0, 512), (512, 256)):
                            nc.tensor.matmul(
                                y[:, o0:o0 + ow],
                                ctx2[qs][:, jj, cs],
                                pw2sb[:, jj, o0:o0 + ow],
                                start=(jj == 0), stop=False)
                    # residual adds ride the PSUM accumulation as
                    # identity-stationary matmuls (vector was the pacer here)
                    for res, last in ((xnr, False), (qres, True)):
                        for o0, ow in ((0, 512), (512, 256)):
                            nc.tensor.matmul(
                                y[:, o0:o0 + ow], ident,
                                res[:, o0:o0 + ow],
                                start=False,
                                stop=(last and ow == 256 and not has_projb))
                    if has_projb:
                        for o0, ow in ((0, 512), (512, 256)):
                            nc.tensor.matmul(
                                y[:, o0:o0 + ow], ones_bf[0:1, :],
                                projb_sb[0:1, o0:o0 + ow],
                                start=False, stop=(ow == 256))
                    ot = s2.tile([P, C], bf16, tag="otb", bufs=3)
                    nc.scalar.copy(ot, y)
                    nc.sync.dma_start(o_t[ks][cs, :], ot)
                    mv, rstd = ln_stats(ot)
                    return (ks, c, ot, mv, rstd)

            def proj_tail(state):
                """LN2 apply + transpose + store, one chunk behind proj_mm."""
                ks, c, ot, mv, rstd = state
                cs = slice(c * P, (c + 1) * P)
                with nc.named_scope(f"proj_{ks}"):
                    x2b = s2.tile([P, C], bf16, tag="x2b", bufs=3)
                    ln_apply(x2b, ot, mv, rstd,
                             n2w_sb if n2_aff else None,
                             n2b_sb if n2_aff else None)
                    tp3 = ps.tile([P, KC, P], bf16, tag="A", name="tp3")
                    for t in range(KC):
                        nc.tensor.transpose(tp3[:, t, :], x2b[:, t * P:(t + 1) * P],
                                            ident)
                    x2ts = s2.tile([P, KC, P], bf16, tag="x2ts", bufs=3)
                    nc.scalar.copy(x2ts, tp3)
                    nc.sync.dma_start(
                        x2T_t[ks][:].rearrange("(t p) n -> p t n", p=P)[:, :, cs],
                        x2ts)

            # MLP state per stream
            mlp_x2h = {}
            mlp_hT = {}

            def mlp_load(s):
                with nc.named_scope(f"mlp_{s}"):
                    x2h = []
                    for nh in range(2):
                        xh = sB.tile([P, KC, 512], f8, tag="x2h", bufs=4,
                                     name=f"x2h_{s}{nh}")
                        nc.gpsimd.dma_start(
                            out=xh,
                            in_=x2T_t[s][:].rearrange("(k p) n -> p k n", p=P)
                            [:, :, nh * 512:(nh + 1) * 512])
                        x2h.append(xh)
                    mlp_x2h[s] = x2h
                    hT = []
                    for nh in range(2):
                        hT.append(big.tile([P, HKC, 512], f8, tag="big",
                                           name=f"hT_{s}{nh}"))
                    mlp_hT[s] = hT

            def fc1_step(kc2, streams="ab"):
                """fc1 at kc2 for given streams."""
                w1k = []
                for kk in range(2):
                    w1t = s3b.tile([P, KC, P], f8, tag="w1k", bufs=3)
                    nc.sync.dma_start(w1t, w1v[:, 2 * kc2 + kk, :, :])
                    w1k.append(w1t)
                for s in streams:
                    x2h = mlp_x2h[s]
                    hT = mlp_hT[s]
                    with nc.named_scope(f"mlp_{s}"):
                        # kk-major: gelu(kk=0) overlaps the kk=1 matmuls
                        accs = [ps.tile([P, 2, 512], f32, tag="A",
                                        name=f"facc{nh}")
                                for nh in range(2)]
                        for kk in range(2):
                            for kp in range(0, KC, 2):
                                for nh in range(2):
                                    nc.tensor.matmul(
                                        accs[nh][:, kk, :],
                                        w1k[kk][:, kp:kp + 2, :],
                                        x2h[nh][:, kp:kp + 2, :],
                                        perf_mode=DR,
                                        start=(kp == 0), stop=(kp == KC - 2))
                            kc = 2 * kc2 + kk
                            for nh in range(2):
                                if has_fc1b:
                                    nc.scalar.activation(
                                        hT[nh][:, kc, :], accs[nh][:, kk, :],
                                        AF.Gelu, bias=fc1b_sb[:, kc:kc + 1],
                                        scale=1.0 / W8SCALE)
                                else:
                                    nc.scalar.activation(
                                        hT[nh][:, kc, :], accs[nh][:, kk, :],
                                        AF.Gelu, scale=1.0 / W8SCALE)

            def fc2_chunk(s, nh, sub):
                hT = mlp_hT[s]
                c = nh * 4 + sub
                cs = slice(c * P, (c + 1) * P)
                with nc.named_scope(f"mlp_{s}"):
                    y = ps.tile([P, C], f32, tag="A", name="yfc2")
                    for kc in range(0, HKC, 2):
                        for o0, ow in ((0, 512), (512, 256)):
                            nc.tensor.matmul(
                                y[:, o0:o0 + ow],
                                hT[nh][:, kc:kc + 2, sub * P:(sub + 1) * P],
                                w2sb[:, kc:kc + 2, o0:o0 + ow],
                                perf_mode=DR,
                                start=(kc == 0),
                                stop=(kc == HKC - 2 and not has_fc2b))
                    if has_fc2b:
                        for o0, ow in ((0, 512), (512, 256)):
                            nc.tensor.matmul(
                                y[:, o0:o0 + ow], ones_bf[0:1, :],
                                fc2b_sb[0:1, o0:o0 + ow],
                                start=False, stop=True)
                    oh = s2.tile([P, C], bf16, tag="oh")
                    nc.sync.dma_start(oh, o_t[s][cs, :])
                    outt = sF.tile([P, C], f32, tag="f32buf", name="outt")
                    nc.vector.scalar_tensor_tensor(
                        outt, y, 1.0 / W8SCALE, oh, ALU.mult, ALU.add)
                    nc.sync.dma_start(out_d[s][cs, :], outt)

            # ---- attention: both dirs merged per head ----
            for j in range(NJ):
                for hp in range(2):
                    attn_head(j, hp)

            # ---- proj both streams, alternating chunks, tail skewed ----
            prev = []
            for c in range(NCH):
                cur = [proj_mm("b", "a", c), proj_mm("a", "b", c)]
                for st in prev:
                    proj_tail(st)
                prev = cur
            for st in prev:
                proj_tail(st)
            mlp_load("a")
            mlp_load("b")
            for kc2 in range(HKC // 2):
                fc1_step(kc2, "a")
            FC2A = {0: [0], 1: [1], 2: [2], 3: [3], 4: [4], 5: [5],
                    6: [6], 7: [7], 8: [], 9: [], 10: [], 11: []}
            for kc2 in range(HKC // 2):
                fc1_step(kc2, "b")
                for idx in FC2A.get(kc2, []):
                    fc2_chunk("a", idx // 4, idx % 4)
            for idx in range(8):
                fc2_chunk("b", idx // 4, idx % 4)

    nc.finalize()
    return nc


def _get_nc(flags):
    if flags not in _CACHE:
        _CACHE[flags] = _build(flags)
    return _CACHE[flags]


def _prep(inputs):
    import ml_dtypes

    f = np.float32
    bf = ml_dtypes.bfloat16
    f8 = ml_dtypes.float8_e4m3
    w = {k: np.asarray(v, f) for k, v in inputs.items()}
    flags = (
        not (np.all(w["norm1_w"] == 1) and np.all(w["norm1_b"] == 0)),
        not (np.all(w["hln_w"] == 1) and np.all(w["hln_b"] == 0)),
        not (np.all(w["norm2_w"] == 1) and np.all(w["norm2_b"] == 0)),
        bool(np.any(w["proj_b"] != 0)),
        bool(np.any(w["fc1_b"] != 0)),
        bool(np.any(w["fc2_b"] != 0)),
    )
    # qkv weights: transpose + fold head-LN centering (linear in x)
    wT = np.ascontiguousarray(w["qkv_w"].T)                   # [C, 3C]
    wT3 = wT.reshape(C, NG, HD)
    wTc = (wT3 - wT3.mean(axis=2, keepdims=True)).reshape(C, S3)
    # proj weights packed by head pair: pw2[p=(h%2)*64+d, j=h//2, o]
    pw = w["proj_w"].T.reshape(NJ, 2, HD, C).transpose(1, 2, 0, 3).reshape(P, NJ * C)
    # fc1 packed: w1p[p, kc, k, f'] = 32*fc1_w[kc*128+f', k*128+p], fp8
    w1p = (W8SCALE * w["fc1_w"]).reshape(HKC, P, KC, P).transpose(3, 0, 2, 1).reshape(P, HKC * C)
    # fc2 packed: w2p[p, kc, o] = 32*fc2_w[o, kc*128+p], fp8
    w2p = (W8SCALE * w["fc2_w"]).reshape(C, HKC, P).transpose(2, 1, 0).reshape(P, HKC * C)
    shared = {
        "qkv_wT": wTc.astype(bf),
        "pw2": np.ascontiguousarray(pw).astype(bf),
        "w1p": np.ascontiguousarray(w1p).astype(f8),
        "w2p": np.ascontiguousarray(w2p).astype(f8),
    }
    n1_aff, hln_aff, n2_aff, pb, f1b, f2b = flags
    if pb:
        shared["proj_b"] = w["proj_b"].reshape(1, C).astype(bf)
    if f1b:
        shared["fc1_b"] = w["fc1_b"]
    if f2b:
        shared["fc2_b"] = (w["fc2_b"] * W8SCALE).reshape(1, C).astype(bf)
    if n1_aff:
        shared["norm1_w"] = w["norm1_w"]
        shared["norm1_b"] = w["norm1_b"]
    if n2_aff:
        shared["norm2_w"] = w["norm2_w"]
        shared["norm2_b"] = w["norm2_b"]
    if hln_aff:
        shared["hln_w"] = w["hln_w"]
        shared["hln_b"] = w["hln_b"]
    return w, flags, shared


def kernel(trace=False, **inputs):
    from concourse.bass_utils import run_bass_kernel_spmd

    w, flags, shared = _prep(inputs)
    nc = _get_nc(flags)
    before = np.ascontiguousarray(w["before"], dtype=np.float32)
    after = np.ascontiguousarray(w["after"], dtype=np.float32)
    in_maps = []
    for core in range(B):
        m = dict(shared)
        m["x_b"] = np.ascontiguousarray(before[core])
        m["x_a"] = np.ascontiguousarray(after[core])
        in_maps.append(m)
    res = run_bass_kernel_spmd(nc, in_maps, core_ids=list(range(B)), trace=trace)
    before_o = np.stack([res.results[i]["out_b"] for i in range(B)])
    after_o = np.stack([res.results[i]["out_a"] for i in range(B)])
    out = (before_o.astype(np.float32), after_o.astype(np.float32))
    if trace:
        return out, res
    return out


# revision 61
# speedup vs baseline: 1.0245x; 1.0020x over previous
"""Trainium2 Bass kernel for nn_Block_22325240004804 (dense_transformer).

Two-stream cross-attention transformer block, B=8 N=1024 C=768 H=12.
Sharding: pure data parallel - batch element b on core b (no collectives).

v8 (on top of v5; 901us -> ~670us):
  - merged-direction attention: per (head, parity) both directions'
    QK matmuls issue as one 8-instruction burst, exps for dir1 overlap
    dir2's QKs, and PV is deferred one mc2 so it never waits on exp
    (PE p-state: only >3us continuous bursts reach max clock; the old
    per-dir chains ran everything at mid/low clock)
  - pt2 layout [P, nh, mi, n]: PV DoubleRow k-tile pair contiguous
  - phase1: LN1 chain issued one chunk ahead of its transposes; QKV
    sections aligned to q/k/v (768 each) and BOTH streams' sections
    interleaved, so ~20us of matmuls cover the per-section stats chains
    (square->reduce->sqrt->recip->normalize) running two-wide on
    vector/scalar; k-transposes stored before q so attention's first
    loads unblock sooner
  - bf16 staging for xn/o residuals (halves staging DMA, kills casts);
    xnT copies on scalar; LN2 + residual adds on vector (scalar is
    exp/gelu-critical); denominator shift DMAs on gpsimd
  - proj software-pipelined: LN2-apply/transpose stage skewed one chunk
    behind matmul/residual; mlp: fc1_a, then fc1_b || fc2_a, then fc2_b
    with kk-major fc1 so gelu(kk=0) overlaps kk=1 matmuls
  - w2 prefetched at attention start (reuses wq's SBUF slot)
  Rejected experimentally: fp8 DoubleRow QK (no DR speedup at mid
  p-state, +LDW cost), fp8 QKV (weight quantization error 2.7e-2 >
  2e-2 gate), gpsimd tensor ops on critical paths (too slow).
"""

import sys

if "/opt/trn_rl_repo" not in sys.path:
    sys.path.insert(0, "/opt/trn_rl_repo")

import numpy as np

B, N, C = 8, 1024, 768
H, HD = 12, 64
S3 = 3 * C          # 2304
HID = 4 * C         # 3072
EPS = 1e-5
P = 128
NCH = N // P        # 8 token chunks
KC = C // P         # 6 contraction chunks over C
NG = S3 // HD       # 36 head-groups per token row
HKC = HID // P      # 24 chunks over HID
NJ = H // 2         # 6 head pairs
W8SCALE = 32.0      # host scale on fp8 mlp weights
ELN16 = -2.772588722239781  # -ln(16): fp8-range shift for exp

_CACHE = {}


def _build(flags):
    import concourse.bass as bass
    import concourse.tile as tile
    from concourse import bacc, mybir

    f32 = mybir.dt.float32
    bf16 = mybir.dt.bfloat16
    f8 = mybir.dt.float8e4
    AF = mybir.ActivationFunctionType
    ALU = mybir.AluOpType
    AX = mybir.AxisListType.X
    DR = mybir.MatmulPerfMode.DoubleRow

    (n1_aff, hln_aff, n2_aff, has_projb, has_fc1b, has_fc2b) = flags

    nc = bacc.Bacc("TRN2", target_bir_lowering=False)

    # ---------------- I/O ----------------
    x_in = {
        "b": nc.dram_tensor("x_b", [N, C], f32, kind="ExternalInput"),
        "a": nc.dram_tensor("x_a", [N, C], f32, kind="ExternalInput"),
    }
    qkv_wT = nc.dram_tensor("qkv_wT", [C, S3], bf16, kind="ExternalInput")
    pw2_d = nc.dram_tensor("pw2", [P, NJ * C], bf16, kind="ExternalInput")
    w1p_d = nc.dram_tensor("w1p", [P, HKC * C], f8, kind="ExternalInput")
    w2p_d = nc.dram_tensor("w2p", [P, HKC * C], f8, kind="ExternalInput")
    projb_d = nc.dram_tensor("proj_b", [1, C], bf16, kind="ExternalInput") if has_projb else None
    fc1b_d = nc.dram_tensor("fc1_b", [HID], f32, kind="ExternalInput") if has_fc1b else None
    fc2b_d = nc.dram_tensor("fc2_b", [1, C], bf16, kind="ExternalInput") if has_fc2b else None
    n1w_d = nc.dram_tensor("norm1_w", [C], f32, kind="ExternalInput") if n1_aff else None
    n1b_d = nc.dram_tensor("norm1_b", [C], f32, kind="ExternalInput") if n1_aff else None
    n2w_d = nc.dram_tensor("norm2_w", [C], f32, kind="ExternalInput") if n2_aff else None
    n2b_d = nc.dram_tensor("norm2_b", [C], f32, kind="ExternalInput") if n2_aff else None
    hlnw_d = nc.dram_tensor("hln_w", [HD], f32, kind="ExternalInput") if hln_aff else None
    hlnb_d = nc.dram_tensor("hln_b", [HD], f32, kind="ExternalInput") if hln_aff else None
    out_d = {
        "b": nc.dram_tensor("out_b", [N, C], f32, kind="ExternalOutput"),
        "a": nc.dram_tensor("out_a", [N, C], f32, kind="ExternalOutput"),
    }

    with tile.TileContext(nc) as tc:
        with (
            tc.tile_pool(name="dram", bufs=1, space="DRAM") as dram,
            tc.tile_pool(name="const", bufs=1) as const,
            tc.tile_pool(name="big", bufs=4) as big,    # xnT/ctx2/hT rotate
            tc.tile_pool(name="s1", bufs=1) as s1,      # weights + va
            tc.tile_pool(name="sB", bufs=2) as sB,
            tc.tile_pool(name="s2", bufs=2) as s2,
            tc.tile_pool(name="sF", bufs=4) as sF,      # f32 chunk buffers
            tc.tile_pool(name="s3", bufs=2) as s3,
            tc.tile_pool(name="s3b", bufs=3) as s3b,
            tc.tile_pool(name="ps", bufs=4, space="PSUM") as ps,
        ):
            # -------- DRAM staging --------
            xn_t = {s: dram.tile([N, C], bf16, name=f"xn_{s}", tag=f"xn_{s}") for s in "ba"}
            qkT_t = {s: dram.tile([2 * C, N], bf16, name=f"qkT_{s}", tag=f"qkT_{s}") for s in "ba"}
            v_t = {s: dram.tile([N, C], f8, name=f"v_{s}", tag=f"v_{s}") for s in "ba"}
            qr_t = {s: dram.tile([H * N, HD], bf16, name=f"qr_{s}", tag=f"qr_{s}") for s in "ba"}
            o_t = {s: dram.tile([N, C], bf16, name=f"o_{s}", tag=f"o_{s}") for s in "ba"}
            x2T_t = {s: dram.tile([C, N], bf16, name=f"x2T_{s}", tag=f"x2T_{s}") for s in "ba"}

            # -------- constants --------
            from concourse.masks import make_identity
            ident = const.tile([P, P], bf16, tag="ident")
            make_identity(nc, ident)
            epsC = const.tile([P, 1], f32, tag="epsC")
            nc.vector.memset(epsC, EPS)
            eln = const.tile([P, 1], f32, tag="eln")
            nc.vector.memset(eln, ELN16)

            if has_projb or has_fc2b:
                ones_bf = const.tile([1, P], bf16, tag="ones_bf")
                nc.vector.memset(ones_bf, 1.0)
            if has_projb:
                projb_sb = const.tile([1, C], bf16, tag="projb")
                nc.sync.dma_start(projb_sb, projb_d[:])
            if has_fc2b:
                fc2b_sb = const.tile([1, C], bf16, tag="fc2b")
                nc.sync.dma_start(fc2b_sb, fc2b_d[:])
            if has_fc1b:
                fc1b_sb = const.tile([P, HKC], f32, tag="fc1b")
                nc.sync.dma_start(fc1b_sb, fc1b_d[:].rearrange("(k p) -> p k", p=P))

            def bcast_load(src_ap, cols, tag):
                t = const.tile([P, cols], f32, tag=tag)
                bc = bass.AP(tensor=src_ap.tensor, offset=src_ap.offset,
                             ap=[[0, P]] + list(src_ap.ap))
                nc.gpsimd.dma_start(out=t, in_=bc)
                return t

            if n1_aff:
                n1w_sb = bcast_load(n1w_d[:], C, "n1w")
                n1b_sb = bcast_load(n1b_d[:], C, "n1b")
            if n2_aff:
                n2w_sb = bcast_load(n2w_d[:], C, "n2w")
                n2b_sb = bcast_load(n2b_d[:], C, "n2b")
            if hln_aff:
                hlnw_sb = bcast_load(hlnw_d[:], HD, "hlnw")
                hlnb_sb = bcast_load(hlnb_d[:], HD, "hlnb")

            # -------- helpers --------
            def ln_stats(x_tile):
                """bn stats over free dim 768 -> (mu [P,1], rstd [P,1])."""
                st = s2.tile([P, 3, 6], f32, tag="lnst", bufs=4)
                for g in range(3):
                    nc.vector.bn_stats(st[:, g, :], x_tile[:, g * 256:(g + 1) * 256])
                mv = s2.tile([P, 2], f32, tag="lnmv", bufs=4)
                nc.vector.bn_aggr(mv, st)
                std = s2.tile([P, 1], f32, tag="lnstd", bufs=4)
                nc.scalar.activation(std, mv[:, 1:2], AF.Sqrt, bias=epsC)
                rstd = s2.tile([P, 1], f32, tag="lnrstd", bufs=4)
                nc.vector.reciprocal(rstd, std)
                return mv, rstd

            def ln_apply(out_tile, x_tile, mv, rstd, w_sb, b_sb):
                nc.vector.tensor_scalar(out_tile, x_tile, mv[:, 0:1], rstd,
                                        ALU.subtract, ALU.mult)
                if w_sb is not None:
                    nc.vector.tensor_tensor(out_tile, out_tile, w_sb, ALU.mult)
                    nc.vector.tensor_tensor(out_tile, out_tile, b_sb, ALU.add)


            # ======== P1 + QKV, streams interleaved per chunk ========
            xnTd = {}
            for s in "ba":
                xnTd[s] = big.tile([P, KC, N], bf16, name=f"xnT_{s}", tag="big")
            wq = s1.tile([P, KC, S3], bf16, tag="wbig")
            nc.scalar.dma_start(wq, qkv_wT[:].rearrange("(k p) f -> p k f", p=P))
            pw2sb = s1.tile([P, NJ, C], bf16, tag="pw2sb")
            nc.scalar.dma_start(pw2sb, pw2_d[:].rearrange("p (j o) -> p j o", o=C))

            xnb_st = {}

            def p1_ln(s, c):
                """LN1 chain (vector) — issued one chunk ahead of p1_tp."""
                cs = slice(c * P, (c + 1) * P)
                with nc.named_scope(f"p1_{s}"):
                    xt = sF.tile([P, C], f32, tag="f32buf", name="xt")
                    nc.sync.dma_start(xt, x_in[s][cs, :])
                    mv, rstd = ln_stats(xt)
                    xnb = s2.tile([P, C], bf16, tag="xnb")
                    ln_apply(xnb, xt, mv, rstd,
                             n1w_sb if n1_aff else None,
                             n1b_sb if n1_aff else None)
                    nc.sync.dma_start(xn_t[s][cs, :], xnb)
                    xnb_st[(s, c)] = xnb

            def p1_tp(s, c):
                cs = slice(c * P, (c + 1) * P)
                xnb = xnb_st.pop((s, c))
                with nc.named_scope(f"p1_{s}"):
                    tp = ps.tile([P, KC, P], bf16, tag="A", name="tp1")
                    for t in range(KC):
                        nc.tensor.transpose(tp[:, t, :], xnb[:, t * P:(t + 1) * P], ident)
                    nc.scalar.copy(xnTd[s][:, :, cs], tp)

            def qkv_chunk2(c):
                """Both streams, section-interleaved: six matmul sections
                (~20us of PE work) cover the per-section stats chains
                (square->reduce->sqrt->recip->normalize) running two-wide
                on vector/scalar, so the q/k transposes never stall PE."""
                cs = slice(c * P, (c + 1) * P)
                zbs = {}
                for s in "ba":
                    with nc.named_scope(f"qkv_{s}"):
                        zbs[s] = s2.tile([P, S3], bf16, tag="zb", name=f"zb_{s}")

                def sect_mm(s, si):
                    f0 = si * C
                    with nc.named_scope(f"qkv_{s}"):
                        acc = ps.tile([P, C], f32, tag="A", name=f"qacc{si}")
                        for k in range(KC):
                            for m0, mw in ((0, 512), (512, 256)):
                                nc.tensor.matmul(
                                    acc[:, m0:m0 + mw],
                                    xnTd[s][:, k, cs],
                                    wq[:, k, f0 + m0:f0 + m0 + mw],
                                    start=(k == 0), stop=(k == KC - 1))
                        return acc

                def sect_norm(s, si, acc):
                    f0 = si * C
                    gw = C // HD  # 12
                    with nc.named_scope(f"qkv_{s}"):
                        sq = s2.tile([P, 1024], bf16, tag="sq")
                        nc.scalar.activation(sq[:, :C], acc, AF.Square)
                        sumsq = s2.tile([P, NG], bf16, tag="hsumsq", bufs=3)
                        with nc.allow_low_precision("head-norm sumsq; rstd "
                                                    "scale err ~0.2%"):
                            nc.vector.reduce_sum(
                                sumsq[:, :gw],
                                sq[:, :C].rearrange("p (g d) -> p g d", d=HD),
                                axis=AX)
                        stdq = s2.tile([P, NG], f32, tag="hstd", bufs=3)
                        nc.scalar.activation(stdq[:, :gw], sumsq[:, :gw],
                                             AF.Sqrt, bias=epsC,
                                             scale=1.0 / HD)
                        rstd = s2.tile([P, NG], f32, tag="hrstd", bufs=3)
                        nc.vector.reciprocal(rstd[:, :gw], stdq[:, :gw])
                        zv = zbs[s][:, f0:f0 + C].rearrange(
                            "p (g d) -> p g d", d=HD)
                        nc.vector.tensor_tensor(
                            zv,
                            acc.rearrange("p (g d) -> p g d", d=HD),
                            rstd[:, :gw, None].to_broadcast([P, gw, HD]),
                            ALU.mult)
                        if hln_aff:
                            nc.vector.tensor_tensor(
                                zv, zv,
                                hlnw_sb[:, None, :].to_broadcast([P, gw, HD]),
                                ALU.mult)
                            nc.vector.tensor_tensor(
                                zv, zv,
                                hlnb_sb[:, None, :].to_broadcast([P, gw, HD]),
                                ALU.add)

                def sect_tp(s, half):
                    with nc.named_scope(f"qkv_{s}"):
                        tp2 = ps.tile([P, KC, P], bf16, tag="A", name="tp2")
                        for t in range(KC):
                            tt = half * KC + t
                            nc.tensor.transpose(
                                tp2[:, t, :], zbs[s][:, tt * P:(tt + 1) * P],
                                ident)
                        qkt_sb = s2.tile([P, KC, P], bf16, tag="qkt", bufs=3)
                        nc.scalar.copy(qkt_sb, tp2)
                        nc.sync.dma_start(
                            qkT_t[s][:].rearrange("(t p) n -> p t n", p=P)
                            [:, half * KC:(half + 1) * KC, cs],
                            qkt_sb)

                acc_q = {s: sect_mm(s, 0) for s in "ba"}
                acc_k = {}
                for s in "ba":
                    acc_k[s] = sect_mm(s, 1)
                for s in "ba":
                    sect_norm(s, 0, acc_q[s])
                acc_v = {}
                for s in "ba":
                    acc_v[s] = sect_mm(s, 2)
                for s in "ba":
                    sect_norm(s, 1, acc_k[s])
                for s in "ba":
                    sect_tp(s, 1)            # k transposes first for attn
                for s in "ba":
                    sect_norm(s, 2, acc_v[s])
                for s in "ba":
                    sect_tp(s, 0)
                for s in "ba":
                    with nc.named_scope(f"qkv_{s}"):
                        nc.gpsimd.dma_start(v_t[s][cs, :], zbs[s][:, 2 * C:])
                        nc.sync.dma_start(
                            qr_t[s][:].rearrange("(h n) d -> n h d", h=H)[cs],
                            zbs[s][:, :C].rearrange("p (g d) -> p g d", d=HD))

            for s in "ba":
                p1_ln(s, 0)
            for c in range(NCH):
                for s in "ba":
                    p1_tp(s, c)
                if c + 1 < NCH:
                    for s in "ba":
                        p1_ln(s, c + 1)
                qkv_chunk2(c)

            # ======== attention + proj + mlp, software-pipelined ========
            DIRS = (("b", "a"), ("a", "b"))  # (qs, ks); output goes to stream ks
            ctx2 = {}
            for qs, ks in DIRS:
                ctx2[qs] = big.tile([P, NJ, N], bf16, name=f"ctx2_{qs}", tag="big")
            # persistent [v | ones] stationaries: parity x direction
            vap = {}
            for hp in range(2):
                for qs, ks in DIRS:
                    t = s1.tile([P, NCH, P], f8, tag=f"va{hp}{qs}")
                    nc.gpsimd.memset(t[:, :, (1 - hp) * HD:(2 - hp) * HD], 1.0)
                    vap[(hp, qs)] = t

            # prefetch mlp fc2 weights (shares the wq slot; frees at attn start)
            w2sb = s1.tile([P, HKC, C], f8, tag="wbig")
            nc.gpsimd.dma_start(w2sb, w2p_d[:].rearrange("p (k o) -> p k o", o=C))
            w1v = w1p_d[:].rearrange("p (kc k f) -> p kc k f", k=KC, f=P)

            def attn_head(j, hp):
                """Both directions merged: QK bursts of 8 back-to-back
                matmuls (>3us: PE ramps to max p-state); PV deferred one
                mc2 so it never waits on exp."""
                h = 2 * j + hp
                hs = slice(hp * HD, (hp + 1) * HD)        # ctx half
                ds = slice((1 - hp) * HD, (2 - hp) * HD)  # denominator half
                lo = slice(0, HD)
                qts, kts, cps = {}, {}, {}
                for qs, ks in DIRS:
                    with nc.named_scope(f"attn_{qs}"):
                        qt = s3b.tile([HD, N], bf16, tag="qh", name=f"qh_{qs}{h}")
                        nc.sync.dma_start(qt, qkT_t[qs][h * HD:(h + 1) * HD, :])
                        kt = s3b.tile([HD, N], bf16, tag="kh", name=f"kh_{qs}{h}")
                        nc.sync.dma_start(
                            kt, qkT_t[ks][C + h * HD:C + (h + 1) * HD, :])
                        va = vap[(hp, qs)]
                        nc.sync.dma_start(
                            va[:, :, hp * HD:(hp + 1) * HD],
                            v_t[ks][:].rearrange("(c p) f -> p c f", p=P)
                            [:, :, h * HD:(h + 1) * HD])
                        qts[qs], kts[qs] = qt, kt
                        cps[qs] = ps.tile([P, 2, 512], f32, tag="A",
                                          name=f"cps_{qs}")
                pend = []  # deferred PV: (qs, mc2, pt2)
                for mc2 in range(NCH // 2):
                    cur = []
                    for qs, ks in DIRS:
                        with nc.named_scope(f"attn_{qs}"):
                            # pt2[p, nh, mi, n]: PV k-tile pair contiguous
                            pt2 = s3b.tile([P, 2, 2, 512], f8, tag="pt",
                                           bufs=4)
                            for mi in range(2):
                                mc = 2 * mc2 + mi
                                sps = ps.tile([P, 2, 512], f32, tag="A",
                                              name="sps")
                                for nh in range(2):
                                    nc.tensor.matmul(
                                        sps[:, nh, :],
                                        kts[qs][:, mc * P:(mc + 1) * P],
                                        qts[qs][:, nh * 512:(nh + 1) * 512])
                                nc.scalar.activation(
                                    pt2[:, :, mi, :], sps,
                                    AF.Exp, scale=float(HD ** -0.5), bias=eln)
                            cur.append((qs, mc2, pt2))
                    for qs, pmc2, pt2 in pend:
                        with nc.named_scope(f"attn_{qs}"):
                            for nh in range(2):
                                nc.tensor.matmul(
                                    cps[qs][:, nh, :],
                                    vap[(hp, qs)][:, 2 * pmc2:2 * pmc2 + 2, :],
                                    pt2[:, nh],
                                    perf_mode=DR,
                                    start=(pmc2 == 0),
                                    stop=(pmc2 == NCH // 2 - 1))
                    pend = cur
                for qs, pmc2, pt2 in pend:
                    with nc.named_scope(f"attn_{qs}"):
                        for nh in range(2):
                            nc.tensor.matmul(
                                cps[qs][:, nh, :],
                                vap[(hp, qs)][:, 2 * pmc2:2 * pmc2 + 2, :],
                                pt2[:, nh],
                                perf_mode=DR,
                                start=(pmc2 == 0),
                                stop=(pmc2 == NCH // 2 - 1))
                for qs, ks in DIRS:
                    with nc.named_scope(f"attn_{qs}"):
                        # denominator (replicated on partitions ds):
                        # aligned copy out of PSUM, shift to base 0,
                        # recipfast at base 0, shift to hs, aligned mult.
                        dn = s3.tile([P, N], f32, tag="dn")
                        nc.vector.tensor_copy(
                            dn[ds, :],
                            cps[qs][ds, :, :].rearrange("p a b -> p (a b)"))
                        if hp == 0:
                            nc.gpsimd.dma_start(dn[lo, :], dn[ds, :])
                        rd = s3.tile([P, N], f32, tag="rd")
                        nc.vector.reciprocal_approx_fast(rd[lo, :], dn[lo, :])
                        if hp == 1:
                            nc.gpsimd.dma_start(rd[hs, :], rd[lo, :])
                        nc.vector.tensor_tensor(
                            ctx2[qs][hs, j, :],
                            cps[qs][hs, :, :].rearrange("p a b -> p (a b)"),
                            rd[hs, :], ALU.mult)

            def proj_mm(qs, ks, c):
                """proj matmul + residual + LN2 stats for chunk c."""
                cs = slice(c * P, (c + 1) * P)
                with nc.named_scope(f"proj_{ks}"):
                    qr_view = qr_t[qs][:].rearrange("(n j) d -> n (j d)", j=H)
                    xnr = s2.tile([P, C], bf16, tag="xnr", bufs=3)
                    nc.sync.dma_start(xnr, xn_t[ks][cs, :])
                    qres = s2.tile([P, C], bf16, tag="qres", bufs=3)
                    nc.sync.dma_start(qres, qr_view[cs, :])
                    y = ps.tile([P, C], f32, tag="A", name="yproj")
                    for jj in range(NJ):
                        for o0, ow in ((0, 512), (512, 256)):
                            nc.tensor.matmul(
                                y[:, o0:o0 + ow],
                                ctx2[qs][:, jj, cs],
                                pw2sb[:, jj, o0:o0 + ow],
                                start=(jj == 0), stop=False)
                    # residual adds ride the PSUM accumulation as
                    # identity-stationary matmuls (vector was the pacer here)
                    for res, last in ((xnr, False), (qres, True)):
                        for o0, ow in ((0, 512), (512, 256)):
                            nc.tensor.matmul(
                                y[:, o0:o0 + ow], ident,
                                res[:, o0:o0 + ow],
                                start=False,
                                stop=(last and ow == 256 and not has_projb))
                    if has_projb:
                        for o0, ow in ((0, 512), (512, 256)):
                            nc.tensor.matmul(
                                y[:, o0:o0 + ow], ones_bf[0:1, :],
                                projb_sb[0:1, o0:o0 + ow],
                                start=False, stop=(ow == 256))
                    ot = s2.tile([P, C], bf16, tag="otb", bufs=3)
                    nc.scalar.copy(ot, y)
                    nc.sync.dma_start(o_t[ks][cs, :], ot)
                    mv, rstd = ln_stats(ot)
                    return (ks, c, ot, mv, rstd)

            def proj_tail(state):
                """LN2 apply + transpose + store, one chunk behind proj_mm."""
                ks, c, ot, mv, rstd = state
                cs = slice(c * P, (c + 1) * P)
                with nc.named_scope(f"proj_{ks}"):
                    x2b = s2.tile([P, C], bf16, tag="x2b", bufs=3)
                    ln_apply(x2b, ot, mv, rstd,
                             n2w_sb if n2_aff else None,
                             n2b_sb if n2_aff else None)
                    tp3 = ps.tile([P, KC, P], bf16, tag="A", name="tp3")
                    for t in range(KC):
                        nc.tensor.transpose(tp3[:, t, :], x2b[:, t * P:(t + 1) * P],
                                            ident)
                    x2ts = s2.tile([P, KC, P], bf16, tag="x2ts", bufs=3)
                    nc.scalar.copy(x2ts, tp3)
                    nc.sync.dma_start(
                        x2T_t[ks][:].rearrange("(t p) n -> p t n", p=P)[:, :, cs],
                        x2ts)

            # MLP state per stream
            mlp_x2h = {}
            mlp_hT = {}

            def mlp_load(s):
                with nc.named_scope(f"mlp_{s}"):
                    x2h = []
                    for nh in range(2):
                        xh = sB.tile([P, KC, 512], f8, tag="x2h", bufs=4,
                                     name=f"x2h_{s}{nh}")
                        nc.gpsimd.dma_start(
                            out=xh,
                            in_=x2T_t[s][:].rearrange("(k p) n -> p k n", p=P)
                            [:, :, nh * 512:(nh + 1) * 512])
                        x2h.append(xh)
                    mlp_x2h[s] = x2h
                    hT = []
                    for nh in range(2):
                        hT.append(big.tile([P, HKC, 512], f8, tag="big",
                                           name=f"hT_{s}{nh}"))
                    mlp_hT[s] = hT

            def fc1_step(kc2, streams="ab"):
                """fc1 at kc2 for given streams."""
                w1k = []
                for kk in range(2):
                    w1t = s3b.tile([P, KC, P], f8, tag="w1k", bufs=3)
                    nc.sync.dma_start(w1t, w1v[:, 2 * kc2 + kk, :, :])
                    w1k.append(w1t)
                for s in streams:
                    x2h = mlp_x2h[s]
                    hT = mlp_hT[s]
                    with nc.named_scope(f"mlp_{s}"):
                        # kk-major: gelu(kk=0) overlaps the kk=1 matmuls
                        accs = [ps.tile([P, 2, 512], f32, tag="A",
                                        name=f"facc{nh}")
                                for nh in range(2)]
                        for kk in range(2):
                            for kp in range(0, KC, 2):
                                for nh in range(2):
                                    nc.tensor.matmul(
                                        accs[nh][:, kk, :],
                                        w1k[kk][:, kp:kp + 2, :],
                                        x2h[nh][:, kp:kp + 2, :],
                                        perf_mode=DR,
                                        start=(kp == 0), stop=(kp == KC - 2))
                            kc = 2 * kc2 + kk
                            for nh in range(2):
                                if has_fc1b:
                                    nc.scalar.activation(
                                        hT[nh][:, kc, :], accs[nh][:, kk, :],
                                        AF.Gelu, bias=fc1b_sb[:, kc:kc + 1],
                                        scale=1.0 / W8SCALE)
                                else:
                                    nc.scalar.activation(
                                        hT[nh][:, kc, :], accs[nh][:, kk, :],
                                        AF.Gelu, scale=1.0 / W8SCALE)

            def fc2_chunk(s, nh, sub):
                hT = mlp_hT[s]
                c = nh * 4 + sub
                cs = slice(c * P, (c + 1) * P)
                with nc.named_scope(f"mlp_{s}"):
                    y = ps.tile([P, C], f32, tag="A", name="yfc2")
                    for kc in range(0, HKC, 2):
                        for o0, ow in ((0, 512), (512, 256)):
                            nc.tensor.matmul(
                                y[:, o0:o0 + ow],
                                hT[nh][:, kc:kc + 2, sub * P:(sub + 1) * P],
                                w2sb[:, kc:kc + 2, o0:o0 + ow],
                                perf_mode=DR,
                                start=(kc == 0),
                                stop=(kc == HKC - 2 and not has_fc2b))
                    if has_fc2b:
                        for o0, ow in ((0, 512), (512, 256)):
                            nc.tensor.matmul(
                                y[:, o0:o0 + ow], ones_bf[0:1, :],
                                fc2b_sb[0:1, o0:o0 + ow],
                                start=False, stop=True)
                    oh = s2.tile([P, C], bf16, tag="oh")
                    nc.sync.dma_start(oh, o_t[s][cs, :])
                    outt = sF.tile([P, C], f32, tag="f32buf", name="outt")
                    nc.vector.scalar_tensor_tensor(
                        outt, y, 1.0 / W8SCALE, oh, ALU.mult, ALU.add)
                    nc.sync.dma_start(out_d[s][cs, :], outt)

            # ---- attention: both dirs merged per head ----
            for j in range(NJ):
                for hp in range(2):
                    attn_head(j, hp)

            # ---- proj both streams, alternating chunks, tail skewed ----
            prev = []
            for c in range(NCH):
                cur = [proj_mm("b", "a", c), proj_mm("a", "b", c)]
                for st in prev:
                    proj_tail(st)
                prev = cur
            for st in prev:
                proj_tail(st)
            mlp_load("a")
            mlp_load("b")
            for kc2 in range(HKC // 2):
                fc1_step(kc2, "a")
            FC2A = {0: [0], 1: [1], 2: [2], 3: [3], 4: [4], 5: [5],
                    6: [6], 7: [7], 8: [], 9: [], 10: [], 11: []}
            for kc2 in range(HKC // 2):
                fc1_step(kc2, "b")
                for idx in FC2A.get(kc2, []):
                    fc2_chunk("a", idx // 4, idx % 4)
            for idx in range(8):
                fc2_chunk("b", idx // 4, idx % 4)

    nc.finalize()
    return nc


def _get_nc(flags):
    if flags not in _CACHE:
        _CACHE[flags] = _build(flags)
    return _CACHE[flags]


def _prep(inputs):
    import ml_dtypes

    f = np.float32
    bf = ml_dtypes.bfloat16
    f8 = ml_dtypes.float8_e4m3
    w = {k: np.asarray(v, f) for k, v in inputs.items()}
    flags = (
        not (np.all(w["norm1_w"] == 1) and np.all(w["norm1_b"] == 0)),
        not (np.all(w["hln_w"] == 1) and np.all(w["hln_b"] == 0)),
        not (np.all(w["norm2_w"] == 1) and np.all(w["norm2_b"] == 0)),
        bool(np.any(w["proj_b"] != 0)),
        bool(np.any(w["fc1_b"] != 0)),
        bool(np.any(w["fc2_b"] != 0)),
    )
    # qkv weights: transpose + fold head-LN centering (linear in x)
    wT = np.ascontiguousarray(w["qkv_w"].T)                   # [C, 3C]
    wT3 = wT.reshape(C, NG, HD)
    wTc = (wT3 - wT3.mean(axis=2, keepdims=True)).reshape(C, S3)
    # proj weights packed by head pair: pw2[p=(h%2)*64+d, j=h//2, o]
    pw = w["proj_w"].T.reshape(NJ, 2, HD, C).transpose(1, 2, 0, 3).reshape(P, NJ * C)
    # fc1 packed: w1p[p, kc, k, f'] = 32*fc1_w[kc*128+f', k*128+p], fp8
    w1p = (W8SCALE * w["fc1_w"]).reshape(HKC, P, KC, P).transpose(3, 0, 2, 1).reshape(P, HKC * C)
    # fc2 packed: w2p[p, kc, o] = 32*fc2_w[o, kc*128+p], fp8
    w2p = (W8SCALE * w["fc2_w"]).reshape(C, HKC, P).transpose(2, 1, 0).reshape(P, HKC * C)
    shared = {
        "qkv_wT": wTc.astype(bf),
        "pw2": np.ascontiguousarray(pw).astype(bf),
        "w1p": np.ascontiguousarray(w1p).astype(f8),
        "w2p": np.ascontiguousarray(w2p).astype(f8),
    }
    n1_aff, hln_aff, n2_aff, pb, f1b, f2b = flags
    if pb:
        shared["proj_b"] = w["proj_b"].reshape(1, C).astype(bf)
    if f1b:
        shared["fc1_b"] = w["fc1_b"]
    if f2b:
        shared["fc2_b"] = (w["fc2_b"] * W8SCALE).reshape(1, C).astype(bf)
    if n1_aff:
        shared["norm1_w"] = w["norm1_w"]
        shared["norm1_b"] = w["norm1_b"]
    if n2_aff:
        shared["norm2_w"] = w["norm2_w"]
        shared["norm2_b"] = w["norm2_b"]
    if hln_aff:
        shared["hln_w"] = w["hln_w"]
        shared["hln_b"] = w["hln_b"]
    return w, flags, shared


def kernel(trace=False, **inputs):
    from concourse.bass_utils import run_bass_kernel_spmd

    w, flags, shared = _prep(inputs)
    nc = _get_nc(flags)
    before = np.ascontiguousarray(w["before"], dtype=np.float32)
    after = np.ascontiguousarray(w["after"], dtype=np.float32)
    in_maps = []
    for core in range(B):
        m = dict(shared)
        m["x_b"] = np.ascontiguousarray(before[core])
        m["x_a"] = np.ascontiguousarray(after[core])
        in_maps.append(m)
    res = run_bass_kernel_spmd(nc, in_maps, core_ids=list(range(B)), trace=trace)
    before_o = np.stack([res.results[i]["out_b"] for i in range(B)])
    after_o = np.stack([res.results[i]["out_a"] for i in range(B)])
    out = (before_o.astype(np.float32), after_o.astype(np.float32))
    if trace:
        return out, res
    return out


# revision 62
# speedup vs baseline: 1.0269x; 1.0023x over previous
"""Trainium2 Bass kernel for nn_Block_22325240004804 (dense_transformer).

Two-stream cross-attention transformer block, B=8 N=1024 C=768 H=12.
Sharding: pure data parallel - batch element b on core b (no collectives).

v9 (on top of v5; 901us -> ~657us):
  - merged-direction attention: per (head, parity) both directions'
    QK matmuls issue as one 8-instruction burst, exps for dir1 overlap
    dir2's QKs, and PV is deferred one mc2 so it never waits on exp
    (PE p-state: only >3us continuous bursts reach max clock; the old
    per-dir chains ran everything at mid/low clock)
  - pt2 layout [P, nh, mi, n]: PV DoubleRow k-tile pair contiguous
  - phase1: LN1 chain issued one chunk ahead of its transposes; QKV
    sections aligned to q/k/v (768 each) and BOTH streams' sections
    interleaved, so ~20us of matmuls cover the per-section stats chains
    (square->reduce->sqrt->recip->normalize) running two-wide on
    vector/scalar; k-transposes stored before q so attention's first
    loads unblock sooner
  - bf16 staging for xn/o residuals (halves staging DMA, kills casts);
    xnT copies on scalar; LN2 + residual adds on vector (scalar is
    exp/gelu-critical); denominator shift DMAs on gpsimd
  - proj software-pipelined: LN2-apply/transpose stage skewed one chunk
    behind matmul/residual; both residual adds ride the proj PSUM
    accumulation as identity-stationary matmuls (vector was the pacer)
    and the y->ot drain is a scalar copy; mlp: fc1_a, then
    fc1_b || fc2_a, then fc2_b with kk-major fc1 so gelu(kk=0)
    overlaps kk=1 matmuls
  - w2 prefetched at attention start (reuses wq's SBUF slot)
  Rejected experimentally: fp8 DoubleRow QK (no DR speedup at mid
  p-state, +LDW cost), fp8 QKV (weight quantization error 2.7e-2 >
  2e-2 gate), gpsimd tensor ops on critical paths (too slow).
"""

import sys

if "/opt/trn_rl_repo" not in sys.path:
    sys.path.insert(0, "/opt/trn_rl_repo")

import numpy as np

B, N, C = 8, 1024, 768
H, HD = 12, 64
S3 = 3 * C          # 2304
HID = 4 * C         # 3072
EPS = 1e-5
P = 128
NCH = N // P        # 8 token chunks
KC = C // P         # 6 contraction chunks over C
NG = S3 // HD       # 36 head-groups per token row
HKC = HID // P      # 24 chunks over HID
NJ = H // 2         # 6 head pairs
W8SCALE = 32.0      # host scale on fp8 mlp weights
ELN16 = -2.772588722239781  # -ln(16): fp8-range shift for exp

_CACHE = {}


def _build(flags):
    import concourse.bass as bass
    import concourse.tile as tile
    from concourse import bacc, mybir

    f32 = mybir.dt.float32
    bf16 = mybir.dt.bfloat16
    f8 = mybir.dt.float8e4
    AF = mybir.ActivationFunctionType
    ALU = mybir.AluOpType
    AX = mybir.AxisListType.X
    DR = mybir.MatmulPerfMode.DoubleRow

    (n1_aff, hln_aff, n2_aff, has_projb, has_fc1b, has_fc2b) = flags

    nc = bacc.Bacc("TRN2", target_bir_lowering=False)

    # ---------------- I/O ----------------
    x_in = {
        "b": nc.dram_tensor("x_b", [N, C], f32, kind="ExternalInput"),
        "a": nc.dram_tensor("x_a", [N, C], f32, kind="ExternalInput"),
    }
    qkv_wT = nc.dram_tensor("qkv_wT", [C, S3], bf16, kind="ExternalInput")
    pw2_d = nc.dram_tensor("pw2", [P, NJ * C], bf16, kind="ExternalInput")
    w1p_d = nc.dram_tensor("w1p", [P, HKC * C], f8, kind="ExternalInput")
    w2p_d = nc.dram_tensor("w2p", [P, HKC * C], f8, kind="ExternalInput")
    projb_d = nc.dram_tensor("proj_b", [1, C], bf16, kind="ExternalInput") if has_projb else None
    fc1b_d = nc.dram_tensor("fc1_b", [HID], f32, kind="ExternalInput") if has_fc1b else None
    fc2b_d = nc.dram_tensor("fc2_b", [1, C], bf16, kind="ExternalInput") if has_fc2b else None
    n1w_d = nc.dram_tensor("norm1_w", [C], f32, kind="ExternalInput") if n1_aff else None
    n1b_d = nc.dram_tensor("norm1_b", [C], f32, kind="ExternalInput") if n1_aff else None
    n2w_d = nc.dram_tensor("norm2_w", [C], f32, kind="ExternalInput") if n2_aff else None
    n2b_d = nc.dram_tensor("norm2_b", [C], f32, kind="ExternalInput") if n2_aff else None
    hlnw_d = nc.dram_tensor("hln_w", [HD], f32, kind="ExternalInput") if hln_aff else None
    hlnb_d = nc.dram_tensor("hln_b", [HD], f32, kind="ExternalInput") if hln_aff else None
    out_d = {
        "b": nc.dram_tensor("out_b", [N, C], f32, kind="ExternalOutput"),
        "a": nc.dram_tensor("out_a", [N, C], f32, kind="ExternalOutput"),
    }

    with tile.TileContext(nc) as tc:
        with (
            tc.tile_pool(name="dram", bufs=1, space="DRAM") as dram,
            tc.tile_pool(name="const", bufs=1) as const,
            tc.tile_pool(name="big", bufs=4) as big,    # xnT/ctx2/hT rotate
            tc.tile_pool(name="s1", bufs=1) as s1,      # weights + va
            tc.tile_pool(name="sB", bufs=2) as sB,
            tc.tile_pool(name="s2", bufs=2) as s2,
            tc.tile_pool(name="sF", bufs=4) as sF,      # f32 chunk buffers
            tc.tile_pool(name="s3", bufs=2) as s3,
            tc.tile_pool(name="s3b", bufs=3) as s3b,
            tc.tile_pool(name="ps", bufs=4, space="PSUM") as ps,
        ):
            # -------- DRAM staging --------
            xn_t = {s: dram.tile([N, C], bf16, name=f"xn_{s}", tag=f"xn_{s}") for s in "ba"}
            qkT_t = {s: dram.tile([2 * C, N], bf16, name=f"qkT_{s}", tag=f"qkT_{s}") for s in "ba"}
            v_t = {s: dram.tile([N, C], f8, name=f"v_{s}", tag=f"v_{s}") for s in "ba"}
            qr_t = {s: dram.tile([H * N, HD], bf16, name=f"qr_{s}", tag=f"qr_{s}") for s in "ba"}
            o_t = {s: dram.tile([N, C], bf16, name=f"o_{s}", tag=f"o_{s}") for s in "ba"}
            x2T_t = {s: dram.tile([C, N], bf16, name=f"x2T_{s}", tag=f"x2T_{s}") for s in "ba"}

            # -------- constants --------
            from concourse.masks import make_identity
            ident = const.tile([P, P], bf16, tag="ident")
            make_identity(nc, ident)
            epsC = const.tile([P, 1], f32, tag="epsC")
            nc.vector.memset(epsC, EPS)
            eln = const.tile([P, 1], f32, tag="eln")
            nc.vector.memset(eln, ELN16)

            if has_projb or has_fc2b:
                ones_bf = const.tile([1, P], bf16, tag="ones_bf")
                nc.vector.memset(ones_bf, 1.0)
            if has_projb:
                projb_sb = const.tile([1, C], bf16, tag="projb")
                nc.sync.dma_start(projb_sb, projb_d[:])
            if has_fc2b:
                fc2b_sb = const.tile([1, C], bf16, tag="fc2b")
                nc.sync.dma_start(fc2b_sb, fc2b_d[:])
            if has_fc1b:
                fc1b_sb = const.tile([P, HKC], f32, tag="fc1b")
                nc.sync.dma_start(fc1b_sb, fc1b_d[:].rearrange("(k p) -> p k", p=P))

            def bcast_load(src_ap, cols, tag):
                t = const.tile([P, cols], f32, tag=tag)
                bc = bass.AP(tensor=src_ap.tensor, offset=src_ap.offset,
                             ap=[[0, P]] + list(src_ap.ap))
                nc.gpsimd.dma_start(out=t, in_=bc)
                return t

            if n1_aff:
                n1w_sb = bcast_load(n1w_d[:], C, "n1w")
                n1b_sb = bcast_load(n1b_d[:], C, "n1b")
            if n2_aff:
                n2w_sb = bcast_load(n2w_d[:], C, "n2w")
                n2b_sb = bcast_load(n2b_d[:], C, "n2b")
            if hln_aff:
                hlnw_sb = bcast_load(hlnw_d[:], HD, "hlnw")
                hlnb_sb = bcast_load(hlnb_d[:], HD, "hlnb")

            # -------- helpers --------
            def ln_stats(x_tile):
                """bn stats over free dim 768 -> (mu [P,1], rstd [P,1])."""
                st = s2.tile([P, 3, 6], f32, tag="lnst", bufs=4)
                for g in range(3):
                    nc.vector.bn_stats(st[:, g, :], x_tile[:, g * 256:(g + 1) * 256])
                mv = s2.tile([P, 2], f32, tag="lnmv", bufs=4)
                nc.vector.bn_aggr(mv, st)
                std = s2.tile([P, 1], f32, tag="lnstd", bufs=4)
                nc.scalar.activation(std, mv[:, 1:2], AF.Sqrt, bias=epsC)
                rstd = s2.tile([P, 1], f32, tag="lnrstd", bufs=4)
                nc.vector.reciprocal(rstd, std)
                return mv, rstd

            def ln_apply(out_tile, x_tile, mv, rstd, w_sb, b_sb):
                nc.vector.tensor_scalar(out_tile, x_tile, mv[:, 0:1], rstd,
                                        ALU.subtract, ALU.mult)
                if w_sb is not None:
                    nc.vector.tensor_tensor(out_tile, out_tile, w_sb, ALU.mult)
                    nc.vector.tensor_tensor(out_tile, out_tile, b_sb, ALU.add)


            # ======== P1 + QKV, streams interleaved per chunk ========
            xnTd = {}
            for s in "ba":
                xnTd[s] = big.tile([P, KC, N], bf16, name=f"xnT_{s}", tag="big")
            wq = s1.tile([P, KC, S3], bf16, tag="wbig")
            nc.scalar.dma_start(wq, qkv_wT[:].rearrange("(k p) f -> p k f", p=P))
            pw2sb = s1.tile([P, NJ, C], bf16, tag="pw2sb")
            nc.scalar.dma_start(pw2sb, pw2_d[:].rearrange("p (j o) -> p j o", o=C))

            xnb_st = {}

            def p1_ln(s, c):
                """LN1 chain (vector) — issued one chunk ahead of p1_tp."""
                cs = slice(c * P, (c + 1) * P)
                with nc.named_scope(f"p1_{s}"):
                    xt = sF.tile([P, C], f32, tag="f32buf", name="xt")
                    nc.sync.dma_start(xt, x_in[s][cs, :])
                    mv, rstd = ln_stats(xt)
                    xnb = s2.tile([P, C], bf16, tag="xnb")
                    ln_apply(xnb, xt, mv, rstd,
                             n1w_sb if n1_aff else None,
                             n1b_sb if n1_aff else None)
                    nc.sync.dma_start(xn_t[s][cs, :], xnb)
                    xnb_st[(s, c)] = xnb

            def p1_tp(s, c):
                cs = slice(c * P, (c + 1) * P)
                xnb = xnb_st.pop((s, c))
                with nc.named_scope(f"p1_{s}"):
                    tp = ps.tile([P, KC, P], bf16, tag="A", name="tp1")
                    for t in range(KC):
                        nc.tensor.transpose(tp[:, t, :], xnb[:, t * P:(t + 1) * P], ident)
                    nc.scalar.copy(xnTd[s][:, :, cs], tp)

            def qkv_chunk2(c):
                """Both streams, section-interleaved: six matmul sections
                (~20us of PE work) cover the per-section stats chains
                (square->reduce->sqrt->recip->normalize) running two-wide
                on vector/scalar, so the q/k transposes never stall PE."""
                cs = slice(c * P, (c + 1) * P)
                zbs = {}
                for s in "ba":
                    with nc.named_scope(f"qkv_{s}"):
                        zbs[s] = s2.tile([P, S3], bf16, tag="zb", name=f"zb_{s}")

                def sect_mm(s, si):
                    f0 = si * C
                    with nc.named_scope(f"qkv_{s}"):
                        acc = ps.tile([P, C], f32, tag="A", name=f"qacc{si}")
                        for k in range(KC):
                            for m0, mw in ((0, 512), (512, 256)):
                                nc.tensor.matmul(
                                    acc[:, m0:m0 + mw],
                                    xnTd[s][:, k, cs],
                                    wq[:, k, f0 + m0:f0 + m0 + mw],
                                    start=(k == 0), stop=(k == KC - 1))
                        return acc

                def sect_norm(s, si, acc):
                    f0 = si * C
                    gw = C // HD  # 12
                    with nc.named_scope(f"qkv_{s}"):
                        sq = s2.tile([P, 1024], bf16, tag="sq")
                        nc.scalar.activation(sq[:, :C], acc, AF.Square)
                        sumsq = s2.tile([P, NG], bf16, tag="hsumsq", bufs=3)
                        with nc.allow_low_precision("head-norm sumsq; rstd "
                                                    "scale err ~0.2%"):
                            nc.vector.reduce_sum(
                                sumsq[:, :gw],
                                sq[:, :C].rearrange("p (g d) -> p g d", d=HD),
                                axis=AX)
                        stdq = s2.tile([P, NG], f32, tag="hstd", bufs=3)
                        nc.scalar.activation(stdq[:, :gw], sumsq[:, :gw],
                                             AF.Sqrt, bias=epsC,
                                             scale=1.0 / HD)
                        rstd = s2.tile([P, NG], f32, tag="hrstd", bufs=3)
                        nc.vector.reciprocal(rstd[:, :gw], stdq[:, :gw])
                        zv = zbs[s][:, f0:f0 + C].rearrange(
                            "p (g d) -> p g d", d=HD)
                        nc.vector.tensor_tensor(
                            zv,
                            acc.rearrange("p (g d) -> p g d", d=HD),
                            rstd[:, :gw, None].to_broadcast([P, gw, HD]),
                            ALU.mult)
                        if hln_aff:
                            nc.vector.tensor_tensor(
                                zv, zv,
                                hlnw_sb[:, None, :].to_broadcast([P, gw, HD]),
                                ALU.mult)
                            nc.vector.tensor_tensor(
                                zv, zv,
                                hlnb_sb[:, None, :].to_broadcast([P, gw, HD]),
                                ALU.add)

                def sect_tp(s, half):
                    with nc.named_scope(f"qkv_{s}"):
                        tp2 = ps.tile([P, KC, P], bf16, tag="A", name="tp2")
                        for t in range(KC):
                            tt = half * KC + t
                            nc.tensor.transpose(
                                tp2[:, t, :], zbs[s][:, tt * P:(tt + 1) * P],
                                ident)
                        qkt_sb = s2.tile([P, KC, P], bf16, tag="qkt", bufs=3)
                        nc.scalar.copy(qkt_sb, tp2)
                        nc.sync.dma_start(
                            qkT_t[s][:].rearrange("(t p) n -> p t n", p=P)
                            [:, half * KC:(half + 1) * KC, cs],
                            qkt_sb)

                acc_q = {s: sect_mm(s, 0) for s in "ba"}
                acc_k = {}
                for s in "ba":
                    acc_k[s] = sect_mm(s, 1)
                for s in "ba":
                    sect_norm(s, 0, acc_q[s])
                acc_v = {}
                for s in "ba":
                    acc_v[s] = sect_mm(s, 2)
                for s in "ba":
                    sect_norm(s, 1, acc_k[s])
                for s in "ba":
                    sect_tp(s, 1)            # k transposes first for attn
                for s in "ba":
                    sect_norm(s, 2, acc_v[s])
                for s in "ba":
                    sect_tp(s, 0)
                for s in "ba":
                    with nc.named_scope(f"qkv_{s}"):
                        nc.gpsimd.dma_start(v_t[s][cs, :], zbs[s][:, 2 * C:])
                        nc.sync.dma_start(
                            qr_t[s][:].rearrange("(h n) d -> n h d", h=H)[cs],
                            zbs[s][:, :C].rearrange("p (g d) -> p g d", d=HD))

            for s in "ba":
                p1_ln(s, 0)
            for c in range(NCH):
                for s in "ba":
                    p1_tp(s, c)
                if c + 1 < NCH:
                    for s in "ba":
                        p1_ln(s, c + 1)
                qkv_chunk2(c)

            # ======== attention + proj + mlp, software-pipelined ========
            DIRS = (("b", "a"), ("a", "b"))  # (qs, ks); output goes to stream ks
            ctx2 = {}
            for qs, ks in DIRS:
                ctx2[qs] = big.tile([P, NJ, N], bf16, name=f"ctx2_{qs}", tag="big")
            # persistent [v | ones] stationaries: parity x direction
            vap = {}
            for hp in range(2):
                for qs, ks in DIRS:
                    t = s1.tile([P, NCH, P], f8, tag=f"va{hp}{qs}")
                    nc.gpsimd.memset(t[:, :, (1 - hp) * HD:(2 - hp) * HD], 1.0)
                    vap[(hp, qs)] = t

            # prefetch mlp fc2 weights (shares the wq slot; frees at attn start)
            w2sb = s1.tile([P, HKC, C], f8, tag="wbig")
            nc.gpsimd.dma_start(w2sb, w2p_d[:].rearrange("p (k o) -> p k o", o=C))
            w1v = w1p_d[:].rearrange("p (kc k f) -> p kc k f", k=KC, f=P)

            def attn_head(j, hp):
                """Both directions merged: QK bursts of 8 back-to-back
                matmuls (>3us: PE ramps to max p-state); PV deferred one
                mc2 so it never waits on exp."""
                h = 2 * j + hp
                hs = slice(hp * HD, (hp + 1) * HD)        # ctx half
                ds = slice((1 - hp) * HD, (2 - hp) * HD)  # denominator half
                lo = slice(0, HD)
                qts, kts, cps = {}, {}, {}
                for qs, ks in DIRS:
                    with nc.named_scope(f"attn_{qs}"):
                        qt = s3b.tile([HD, N], bf16, tag="qh", name=f"qh_{qs}{h}")
                        nc.sync.dma_start(qt, qkT_t[qs][h * HD:(h + 1) * HD, :])
                        kt = s3b.tile([HD, N], bf16, tag="kh", name=f"kh_{qs}{h}")
                        nc.sync.dma_start(
                            kt, qkT_t[ks][C + h * HD:C + (h + 1) * HD, :])
                        va = vap[(hp, qs)]
                        nc.sync.dma_start(
                            va[:, :, hp * HD:(hp + 1) * HD],
                            v_t[ks][:].rearrange("(c p) f -> p c f", p=P)
                            [:, :, h * HD:(h + 1) * HD])
                        qts[qs], kts[qs] = qt, kt
                        cps[qs] = ps.tile([P, 2, 512], f32, tag="A",
                                          name=f"cps_{qs}")
                pend = []  # deferred PV: (qs, mc2, pt2)
                for mc2 in range(NCH // 2):
                    cur = []
                    for qs, ks in DIRS:
                        with nc.named_scope(f"attn_{qs}"):
                            # pt2[p, nh, mi, n]: PV k-tile pair contiguous
                            pt2 = s3b.tile([P, 2, 2, 512], f8, tag="pt",
                                           bufs=4)
                            for mi in range(2):
                                mc = 2 * mc2 + mi
                                sps = ps.tile([P, 2, 512], f32, tag="A",
                                              name="sps")
                                for nh in range(2):
                                    nc.tensor.matmul(
                                        sps[:, nh, :],
                                        kts[qs][:, mc * P:(mc + 1) * P],
                                        qts[qs][:, nh * 512:(nh + 1) * 512])
                                nc.scalar.activation(
                                    pt2[:, :, mi, :], sps,
                                    AF.Exp, scale=float(HD ** -0.5), bias=eln)
                            cur.append((qs, mc2, pt2))
                    for qs, pmc2, pt2 in pend:
                        with nc.named_scope(f"attn_{qs}"):
                            for nh in range(2):
                                nc.tensor.matmul(
                                    cps[qs][:, nh, :],
                                    vap[(hp, qs)][:, 2 * pmc2:2 * pmc2 + 2, :],
                                    pt2[:, nh],
                                    perf_mode=DR,
                                    start=(pmc2 == 0),
                                    stop=(pmc2 == NCH // 2 - 1))
                    pend = cur
                for qs, pmc2, pt2 in pend:
                    with nc.named_scope(f"attn_{qs}"):
                        for nh in range(2):
                            nc.tensor.matmul(
                                cps[qs][:, nh, :],
                                vap[(hp, qs)][:, 2 * pmc2:2 * pmc2 + 2, :],
                                pt2[:, nh],
                                perf_mode=DR,
                                start=(pmc2 == 0),
                                stop=(pmc2 == NCH // 2 - 1))
                for qs, ks in DIRS:
                    with nc.named_scope(f"attn_{qs}"):
                        # denominator (replicated on partitions ds):
                        # aligned copy out of PSUM, shift to base 0,
                        # recipfast at base 0, shift to hs, aligned mult.
                        dn = s3.tile([P, N], f32, tag="dn")
                        nc.vector.tensor_copy(
                            dn[ds, :],
                            cps[qs][ds, :, :].rearrange("p a b -> p (a b)"))
                        if hp == 0:
                            nc.gpsimd.dma_start(dn[lo, :], dn[ds, :])
                        rd = s3.tile([P, N], f32, tag="rd")
                        nc.vector.reciprocal_approx_fast(rd[lo, :], dn[lo, :])
                        if hp == 1:
                            nc.gpsimd.dma_start(rd[hs, :], rd[lo, :])
                        nc.vector.tensor_tensor(
                            ctx2[qs][hs, j, :],
                            cps[qs][hs, :, :].rearrange("p a b -> p (a b)"),
                            rd[hs, :], ALU.mult)

            def proj_mm(qs, ks, c):
                """proj matmul + residual + LN2 stats for chunk c."""
                cs = slice(c * P, (c + 1) * P)
                with nc.named_scope(f"proj_{ks}"):
                    qr_view = qr_t[qs][:].rearrange("(n j) d -> n (j d)", j=H)
                    xnr = s2.tile([P, C], bf16, tag="xnr", bufs=3)
                    nc.sync.dma_start(xnr, xn_t[ks][cs, :])
                    qres = s2.tile([P, C], bf16, tag="qres", bufs=3)
                    nc.sync.dma_start(qres, qr_view[cs, :])
                    y = ps.tile([P, C], f32, tag="A", name="yproj")
                    for jj in range(NJ):
                        for o0, ow in ((0, 512), (512, 256)):
                            nc.tensor.matmul(
                                y[:, o0:o0 + ow],
                                ctx2[qs][:, jj, cs],
                                pw2sb[:, jj, o0:o0 + ow],
                                start=(jj == 0), stop=False)
                    # residual adds ride the PSUM accumulation as
                    # identity-stationary matmuls (vector was the pacer here)
                    for res, last in ((xnr, False), (qres, True)):
                        for o0, ow in ((0, 512), (512, 256)):
                            nc.tensor.matmul(
                                y[:, o0:o0 + ow], ident,
                                res[:, o0:o0 + ow],
                                start=False,
                                stop=(last and ow == 256 and not has_projb))
                    if has_projb:
                        for o0, ow in ((0, 512), (512, 256)):
                            nc.tensor.matmul(
                                y[:, o0:o0 + ow], ones_bf[0:1, :],
                                projb_sb[0:1, o0:o0 + ow],
                                start=False, stop=(ow == 256))
                    ot = s2.tile([P, C], bf16, tag="otb", bufs=3)
                    nc.scalar.copy(ot, y)
                    nc.sync.dma_start(o_t[ks][cs, :], ot)
                    mv, rstd = ln_stats(ot)
                    return (ks, c, ot, mv, rstd)

            def proj_tail(state):
                """LN2 apply + transpose + store, one chunk behind proj_mm."""
                ks, c, ot, mv, rstd = state
                cs = slice(c * P, (c + 1) * P)
                with nc.named_scope(f"proj_{ks}"):
                    x2b = s2.tile([P, C], bf16, tag="x2b", bufs=3)
                    ln_apply(x2b, ot, mv, rstd,
                             n2w_sb if n2_aff else None,
                             n2b_sb if n2_aff else None)
                    tp3 = ps.tile([P, KC, P], bf16, tag="A", name="tp3")
                    for t in range(KC):
                        nc.tensor.transpose(tp3[:, t, :], x2b[:, t * P:(t + 1) * P],
                                            ident)
                    x2ts = s2.tile([P, KC, P], bf16, tag="x2ts", bufs=3)
                    nc.scalar.copy(x2ts, tp3)
                    nc.sync.dma_start(
                        x2T_t[ks][:].rearrange("(t p) n -> p t n", p=P)[:, :, cs],
                        x2ts)

            # MLP state per stream
            mlp_x2h = {}
            mlp_hT = {}

            def mlp_load(s):
                with nc.named_scope(f"mlp_{s}"):
                    x2h = []
                    for nh in range(2):
                        xh = sB.tile([P, KC, 512], f8, tag="x2h", bufs=4,
                                     name=f"x2h_{s}{nh}")
                        nc.gpsimd.dma_start(
                            out=xh,
                            in_=x2T_t[s][:].rearrange("(k p) n -> p k n", p=P)
                            [:, :, nh * 512:(nh + 1) * 512])
                        x2h.append(xh)
                    mlp_x2h[s] = x2h
                    hT = []
                    for nh in range(2):
                        hT.append(big.tile([P, HKC, 512], f8, tag="big",
                                           name=f"hT_{s}{nh}"))
                    mlp_hT[s] = hT

            def fc1_step(kc2, streams="ab"):
                """fc1 at kc2 for given streams."""
                w1k = []
                for kk in range(2):
                    w1t = s3b.tile([P, KC, P], f8, tag="w1k", bufs=3)
                    nc.sync.dma_start(w1t, w1v[:, 2 * kc2 + kk, :, :])
                    w1k.append(w1t)
                for s in streams:
                    x2h = mlp_x2h[s]
                    hT = mlp_hT[s]
                    with nc.named_scope(f"mlp_{s}"):
                        # kk-major: gelu(kk=0) overlaps the kk=1 matmuls
                        accs = [ps.tile([P, 2, 512], f32, tag="A",
                                        name=f"facc{nh}")
                                for nh in range(2)]
                        for kk in range(2):
                            for kp in range(0, KC, 2):
                                for nh in range(2):
                                    nc.tensor.matmul(
                                        accs[nh][:, kk, :],
                                        w1k[kk][:, kp:kp + 2, :],
                                        x2h[nh][:, kp:kp + 2, :],
                                        perf_mode=DR,
                                        start=(kp == 0), stop=(kp == KC - 2))
                            kc = 2 * kc2 + kk
                            for nh in range(2):
                                if has_fc1b:
                                    nc.scalar.activation(
                                        hT[nh][:, kc, :], accs[nh][:, kk, :],
                                        AF.Gelu, bias=fc1b_sb[:, kc:kc + 1],
                                        scale=1.0 / W8SCALE)
                                else:
                                    nc.scalar.activation(
                                        hT[nh][:, kc, :], accs[nh][:, kk, :],
                                        AF.Gelu, scale=1.0 / W8SCALE)

            def fc2_chunk(s, nh, sub):
                hT = mlp_hT[s]
                c = nh * 4 + sub
                cs = slice(c * P, (c + 1) * P)
                with nc.named_scope(f"mlp_{s}"):
                    y = ps.tile([P, C], f32, tag="A", name="yfc2")
                    for kc in range(0, HKC, 2):
                        for o0, ow in ((0, 512), (512, 256)):
                            nc.tensor.matmul(
                                y[:, o0:o0 + ow],
                                hT[nh][:, kc:kc + 2, sub * P:(sub + 1) * P],
                                w2sb[:, kc:kc + 2, o0:o0 + ow],
                                perf_mode=DR,
                                start=(kc == 0),
                                stop=(kc == HKC - 2 and not has_fc2b))
                    if has_fc2b:
                        for o0, ow in ((0, 512), (512, 256)):
                            nc.tensor.matmul(
                                y[:, o0:o0 + ow], ones_bf[0:1, :],
                                fc2b_sb[0:1, o0:o0 + ow],
                                start=False, stop=True)
                    oh = s2.tile([P, C], bf16, tag="oh")
                    nc.sync.dma_start(oh, o_t[s][cs, :])
                    outt = sF.tile([P, C], f32, tag="f32buf", name="outt")
                    nc.vector.scalar_tensor_tensor(
                        outt, y, 1.0 / W8SCALE, oh, ALU.mult, ALU.add)
                    nc.sync.dma_start(out_d[s][cs, :], outt)

            # ---- attention: both dirs merged per head ----
            for j in range(NJ):
                for hp in range(2):
                    attn_head(j, hp)

            # ---- proj both streams, alternating chunks, tail skewed ----
            prev = []
            for c in range(NCH):
                cur = [proj_mm("b", "a", c), proj_mm("a", "b", c)]
                for st in prev:
                    proj_tail(st)
                prev = cur
            for st in prev:
                proj_tail(st)
            mlp_load("a")
            mlp_load("b")
            for kc2 in range(HKC // 2):
                fc1_step(kc2, "a")
            FC2A = {0: [0], 1: [1], 2: [2], 3: [3], 4: [4], 5: [5],
                    6: [6], 7: [7], 8: [], 9: [], 10: [], 11: []}
            for kc2 in range(HKC // 2):
                fc1_step(kc2, "b")
                for idx in FC2A.get(kc2, []):
                    fc2_chunk("a", idx // 4, idx % 4)
            for idx in range(8):
                fc2_chunk("b", idx // 4, idx % 4)

    nc.finalize()
    return nc


def _get_nc(flags):
    if flags not in _CACHE:
        _CACHE[flags] = _build(flags)
    return _CACHE[flags]


def _prep(inputs):
    import ml_dtypes

    f = np.float32
    bf = ml_dtypes.bfloat16
    f8 = ml_dtypes.float8_e4m3
    w = {k: np.asarray(v, f) for k, v in inputs.items()}
    flags = (
        not (np.all(w["norm1_w"] == 1) and np.all(w["norm1_b"] == 0)),
        not (np.all(w["hln_w"] == 1) and np.all(w["hln_b"] == 0)),
        not (np.all(w["norm2_w"] == 1) and np.all(w["norm2_b"] == 0)),
        bool(np.any(w["proj_b"] != 0)),
        bool(np.any(w["fc1_b"] != 0)),
        bool(np.any(w["fc2_b"] != 0)),
    )
    # qkv weights: transpose + fold head-LN centering (linear in x)
    wT = np.ascontiguousarray(w["qkv_w"].T)                   # [C, 3C]
    wT3 = wT.reshape(C, NG, HD)
    wTc = (wT3 - wT3.mean(axis=2, keepdims=True)).reshape(C, S3)
    # proj weights packed by head pair: pw2[p=(h%2)*64+d, j=h//2, o]
    pw = w["proj_w"].T.reshape(NJ, 2, HD, C).transpose(1, 2, 0, 3).reshape(P, NJ * C)
    # fc1 packed: w1p[p, kc, k, f'] = 32*fc1_w[kc*128+f', k*128+p], fp8
    w1p = (W8SCALE * w["fc1_w"]).reshape(HKC, P, KC, P).transpose(3, 0, 2, 1).reshape(P, HKC * C)
    # fc2 packed: w2p[p, kc, o] = 32*fc2_w[o, kc*128+p], fp8
    w2p = (W8SCALE * w["fc2_w"]).reshape(C, HKC, P).transpose(2, 1, 0).reshape(P, HKC * C)
    shared = {
        "qkv_wT": wTc.astype(bf),
        "pw2": np.ascontiguousarray(pw).astype(bf),
        "w1p": np.ascontiguousarray(w1p).astype(f8),
        "w2p": np.ascontiguousarray(w2p).astype(f8),
    }
    n1_aff, hln_aff, n2_aff, pb, f1b, f2b = flags
    if pb:
        shared["proj_b"] = w["proj_b"].reshape(1, C).astype(bf)
    if f1b:
        shared["fc1_b"] = w["fc1_b"]
    if f2b:
        shared["fc2_b"] = (w["fc2_b"] * W8SCALE).reshape(1, C).astype(bf)
    if n1_aff:
        shared["norm1_w"] = w["norm1_w"]
        shared["norm1_b"] = w["norm1_b"]
    if n2_aff:
        shared["norm2_w"] = w["norm2_w"]
        shared["norm2_b"] = w["norm2_b"]
    if hln_aff:
        shared["hln_w"] = w["hln_w"]
        shared["hln_b"] = w["hln_b"]
    return w, flags, shared


def kernel(trace=False, **inputs):
    from concourse.bass_utils import run_bass_kernel_spmd

    w, flags, shared = _prep(inputs)
    nc = _get_nc(flags)
    before = np.ascontiguousarray(w["before"], dtype=np.float32)
    after = np.ascontiguousarray(w["after"], dtype=np.float32)
    in_maps = []
    for core in range(B):
        m = dict(shared)
        m["x_b"] = np.ascontiguousarray(before[core])
        m["x_a"] = np.ascontiguousarray(after[core])
        in_maps.append(m)
    res = run_bass_kernel_spmd(nc, in_maps, core_ids=list(range(B)), trace=trace)
    before_o = np.stack([res.results[i]["out_b"] for i in range(B)])
    after_o = np.stack([res.results[i]["out_a"] for i in range(B)])
    out = (before_o.astype(np.float32), after_o.astype(np.float32))
    if trace:
        return out, res
    return out


# revision 63
# speedup vs baseline: 1.0480x; 1.0206x over previous
"""Trainium2 Bass kernel for nn_Block_22325240004804 (dense_transformer).

Two-stream cross-attention transformer block, B=8 N=1024 C=768 H=12.
Sharding: pure data parallel - batch element b on core b (no collectives).

v9 (on top of v5; 901us -> ~657us):
  - merged-direction attention: per (head, parity) both directions'
    QK matmuls issue as one 8-instruction burst, exps for dir1 overlap
    dir2's QKs, and PV is deferred one mc2 so it never waits on exp
    (PE p-state: only >3us continuous bursts reach max clock; the old
    per-dir chains ran everything at mid/low clock)
  - pt2 layout [P, nh, mi, n]: PV DoubleRow k-tile pair contiguous
  - phase1: LN1 chain issued one chunk ahead of its transposes; QKV
    sections aligned to q/k/v (768 each) and BOTH streams' sections
    interleaved, so ~20us of matmuls cover the per-section stats chains
    (square->reduce->sqrt->recip->normalize) running two-wide on
    vector/scalar; k-transposes stored before q so attention's first
    loads unblock sooner
  - bf16 staging for xn/o residuals (halves staging DMA, kills casts);
    xnT copies on scalar; LN2 + residual adds on vector (scalar is
    exp/gelu-critical); denominator shift DMAs on gpsimd
  - proj software-pipelined: LN2-apply/transpose stage skewed one chunk
    behind matmul/residual; both residual adds ride the proj PSUM
    accumulation as identity-stationary matmuls (vector was the pacer)
    and the y->ot drain is a scalar copy; mlp: fc1_a, then
    fc1_b || fc2_a, then fc2_b with kk-major fc1 so gelu(kk=0)
    overlaps kk=1 matmuls
  - w2 prefetched at attention start (reuses wq's SBUF slot)
  Rejected experimentally: fp8 DoubleRow QK (no DR speedup at mid
  p-state, +LDW cost), fp8 QKV (weight quantization error 2.7e-2 >
  2e-2 gate), gpsimd tensor ops on critical paths (too slow).
"""

import sys

if "/opt/trn_rl_repo" not in sys.path:
    sys.path.insert(0, "/opt/trn_rl_repo")

import numpy as np

B, N, C = 8, 1024, 768
H, HD = 12, 64
S3 = 3 * C          # 2304
HID = 4 * C         # 3072
EPS = 1e-5
P = 128
NCH = N // P        # 8 token chunks
KC = C // P         # 6 contraction chunks over C
NG = S3 // HD       # 36 head-groups per token row
HKC = HID // P      # 24 chunks over HID
NJ = H // 2         # 6 head pairs
W8SCALE = 32.0      # host scale on fp8 mlp weights
ELN16 = -2.772588722239781  # -ln(16): fp8-range shift for exp

_CACHE = {}


def _build(flags):
    import concourse.bass as bass
    import concourse.tile as tile
    from concourse import bacc, mybir

    f32 = mybir.dt.float32
    bf16 = mybir.dt.bfloat16
    f8 = mybir.dt.float8e4
    AF = mybir.ActivationFunctionType
    ALU = mybir.AluOpType
    AX = mybir.AxisListType.X
    DR = mybir.MatmulPerfMode.DoubleRow

    (n1_aff, hln_aff, n2_aff, has_projb, has_fc1b, has_fc2b) = flags

    nc = bacc.Bacc("TRN2", target_bir_lowering=False)

    # ---------------- I/O ----------------
    x_in = {
        "b": nc.dram_tensor("x_b", [N, C], f32, kind="ExternalInput"),
        "a": nc.dram_tensor("x_a", [N, C], f32, kind="ExternalInput"),
    }
    qkv_wT = nc.dram_tensor("qkv_wT", [C, S3], bf16, kind="ExternalInput")
    pw2_d = nc.dram_tensor("pw2", [P, NJ * C], bf16, kind="ExternalInput")
    w1p_d = nc.dram_tensor("w1p", [P, HKC * C], f8, kind="ExternalInput")
    w2p_d = nc.dram_tensor("w2p", [P, HKC * C], f8, kind="ExternalInput")
    projb_d = nc.dram_tensor("proj_b", [1, C], bf16, kind="ExternalInput") if has_projb else None
    fc1b_d = nc.dram_tensor("fc1_b", [HID], f32, kind="ExternalInput") if has_fc1b else None
    fc2b_d = nc.dram_tensor("fc2_b", [1, C], bf16, kind="ExternalInput") if has_fc2b else None
    n1w_d = nc.dram_tensor("norm1_w", [C], f32, kind="ExternalInput") if n1_aff else None
    n1b_d = nc.dram_tensor("norm1_b", [C], f32, kind="ExternalInput") if n1_aff else None
    n2w_d = nc.dram_tensor("norm2_w", [C], f32, kind="ExternalInput") if n2_aff else None
    n2b_d = nc.dram_tensor("norm2_b", [C], f32, kind="ExternalInput") if n2_aff else None
    hlnw_d = nc.dram_tensor("hln_w", [HD], f32, kind="ExternalInput") if hln_aff else None
    hlnb_d = nc.dram_tensor("hln_b", [HD], f32, kind="ExternalInput") if hln_aff else None
    out_d = {
        "b": nc.dram_tensor("out_b", [N, C], f32, kind="ExternalOutput"),
        "a": nc.dram_tensor("out_a", [N, C], f32, kind="ExternalOutput"),
    }

    with tile.TileContext(nc) as tc:
        with (
            tc.tile_pool(name="dram", bufs=1, space="DRAM") as dram,
            tc.tile_pool(name="const", bufs=1) as const,
            tc.tile_pool(name="big", bufs=4) as big,    # xnT/ctx2/hT rotate
            tc.tile_pool(name="s1", bufs=1) as s1,      # weights + va
            tc.tile_pool(name="sB", bufs=2) as sB,
            tc.tile_pool(name="s2", bufs=2) as s2,
            tc.tile_pool(name="sF", bufs=4) as sF,      # f32 chunk buffers
            tc.tile_pool(name="s3", bufs=2) as s3,
            tc.tile_pool(name="s3b", bufs=3) as s3b,
            tc.tile_pool(name="ps", bufs=4, space="PSUM") as ps,
        ):
            # -------- DRAM staging --------
            xn_t = {s: dram.tile([N, C], bf16, name=f"xn_{s}", tag=f"xn_{s}") for s in "ba"}
            qkT_t = {s: dram.tile([2 * C, N], bf16, name=f"qkT_{s}", tag=f"qkT_{s}") for s in "ba"}
            v_t = {s: dram.tile([N, C], f8, name=f"v_{s}", tag=f"v_{s}") for s in "ba"}
            qr_t = {s: dram.tile([H * N, HD], bf16, name=f"qr_{s}", tag=f"qr_{s}") for s in "ba"}
            o_t = {s: dram.tile([N, C], bf16, name=f"o_{s}", tag=f"o_{s}") for s in "ba"}
            x2T_t = {s: dram.tile([C, N], bf16, name=f"x2T_{s}", tag=f"x2T_{s}") for s in "ba"}

            # -------- constants --------
            from concourse.masks import make_identity
            ident = const.tile([P, P], bf16, tag="ident")
            make_identity(nc, ident)
            epsC = const.tile([P, 1], f32, tag="epsC")
            nc.vector.memset(epsC, EPS)
            eln = const.tile([P, 1], f32, tag="eln")
            nc.vector.memset(eln, ELN16)

            if has_projb or has_fc2b:
                ones_bf = const.tile([1, P], bf16, tag="ones_bf")
                nc.vector.memset(ones_bf, 1.0)
            if has_projb:
                projb_sb = const.tile([1, C], bf16, tag="projb")
                nc.sync.dma_start(projb_sb, projb_d[:])
            if has_fc2b:
                fc2b_sb = const.tile([1, C], bf16, tag="fc2b")
                nc.sync.dma_start(fc2b_sb, fc2b_d[:])
            if has_fc1b:
                fc1b_sb = const.tile([P, HKC], f32, tag="fc1b")
                nc.sync.dma_start(fc1b_sb, fc1b_d[:].rearrange("(k p) -> p k", p=P))

            def bcast_load(src_ap, cols, tag):
                t = const.tile([P, cols], f32, tag=tag)
                bc = bass.AP(tensor=src_ap.tensor, offset=src_ap.offset,
                             ap=[[0, P]] + list(src_ap.ap))
                nc.gpsimd.dma_start(out=t, in_=bc)
                return t

            if n1_aff:
                n1w_sb = bcast_load(n1w_d[:], C, "n1w")
                n1b_sb = bcast_load(n1b_d[:], C, "n1b")
            if n2_aff:
                n2w_sb = bcast_load(n2w_d[:], C, "n2w")
                n2b_sb = bcast_load(n2b_d[:], C, "n2b")
            if hln_aff:
                hlnw_sb = bcast_load(hlnw_d[:], HD, "hlnw")
                hlnb_sb = bcast_load(hlnb_d[:], HD, "hlnb")

            # -------- helpers --------
            def ln_stats(x_tile):
                """bn stats over free dim 768 -> (mu [P,1], rstd [P,1])."""
                st = s2.tile([P, 3, 6], f32, tag="lnst", bufs=4)
                for g in range(3):
                    nc.vector.bn_stats(st[:, g, :], x_tile[:, g * 256:(g + 1) * 256])
                mv = s2.tile([P, 2], f32, tag="lnmv", bufs=4)
                nc.vector.bn_aggr(mv, st)
                std = s2.tile([P, 1], f32, tag="lnstd", bufs=4)
                nc.scalar.activation(std, mv[:, 1:2], AF.Sqrt, bias=epsC)
                rstd = s2.tile([P, 1], f32, tag="lnrstd", bufs=4)
                nc.vector.reciprocal(rstd, std)
                return mv, rstd

            def ln_apply(out_tile, x_tile, mv, rstd, w_sb, b_sb):
                nc.vector.tensor_scalar(out_tile, x_tile, mv[:, 0:1], rstd,
                                        ALU.subtract, ALU.mult)
                if w_sb is not None:
                    nc.vector.tensor_tensor(out_tile, out_tile, w_sb, ALU.mult)
                    nc.vector.tensor_tensor(out_tile, out_tile, b_sb, ALU.add)


            # ======== P1 + QKV, streams interleaved per chunk ========
            xnTd = {}
            for s in "ba":
                xnTd[s] = big.tile([P, KC, N], bf16, name=f"xnT_{s}", tag="big")
            wq = s1.tile([P, KC, S3], bf16, tag="wbig")
            nc.scalar.dma_start(wq, qkv_wT[:].rearrange("(k p) f -> p k f", p=P))
            pw2sb = s1.tile([P, NJ, C], bf16, tag="pw2sb")
            nc.scalar.dma_start(pw2sb, pw2_d[:].rearrange("p (j o) -> p j o", o=C))

            xnb_st = {}

            def p1_ln(s, c):
                """LN1 chain (vector) — issued one chunk ahead of p1_tp."""
                cs = slice(c * P, (c + 1) * P)
                with nc.named_scope(f"p1_{s}"):
                    xt = sF.tile([P, C], f32, tag="f32buf", name="xt")
                    nc.sync.dma_start(xt, x_in[s][cs, :])
                    mv, rstd = ln_stats(xt)
                    xnb = s2.tile([P, C], bf16, tag="xnb")
                    ln_apply(xnb, xt, mv, rstd,
                             n1w_sb if n1_aff else None,
                             n1b_sb if n1_aff else None)
                    nc.sync.dma_start(xn_t[s][cs, :], xnb)
                    xnb_st[(s, c)] = xnb

            def p1_tp(s, c):
                cs = slice(c * P, (c + 1) * P)
                xnb = xnb_st.pop((s, c))
                with nc.named_scope(f"p1_{s}"):
                    tp = ps.tile([P, KC, P], bf16, tag="A", name="tp1")
                    for t in range(KC):
                        nc.tensor.transpose(tp[:, t, :], xnb[:, t * P:(t + 1) * P], ident)
                    nc.scalar.copy(xnTd[s][:, :, cs], tp)

            def qkv_chunk2(c):
                """Both streams, section-interleaved: six matmul sections
                (~20us of PE work) cover the per-section stats chains
                (square->reduce->sqrt->recip->normalize) running two-wide
                on vector/scalar, so the q/k transposes never stall PE."""
                cs = slice(c * P, (c + 1) * P)
                zbs = {}
                for s in "ba":
                    with nc.named_scope(f"qkv_{s}"):
                        zbs[s] = s2.tile([P, S3], bf16, tag="zb", name=f"zb_{s}")

                def sect_mm(s, si):
                    f0 = si * C
                    with nc.named_scope(f"qkv_{s}"):
                        acc = ps.tile([P, C], f32, tag="A", name=f"qacc{si}")
                        for k in range(KC):
                            for m0, mw in ((0, 512), (512, 256)):
                                nc.tensor.matmul(
                                    acc[:, m0:m0 + mw],
                                    xnTd[s][:, k, cs],
                                    wq[:, k, f0 + m0:f0 + m0 + mw],
                                    start=(k == 0), stop=(k == KC - 1))
                        return acc

                def sect_norm(s, si, acc):
                    f0 = si * C
                    gw = C // HD  # 12
                    with nc.named_scope(f"qkv_{s}"):
                        sq = s2.tile([P, 1024], bf16, tag="sq")
                        nc.scalar.activation(sq[:, :C], acc, AF.Square)
                        sumsq = s2.tile([P, NG], bf16, tag="hsumsq", bufs=3)
                        with nc.allow_low_precision("head-norm sumsq; rstd "
                                                    "scale err ~0.2%"):
                            nc.vector.reduce_sum(
                                sumsq[:, :gw],
                                sq[:, :C].rearrange("p (g d) -> p g d", d=HD),
                                axis=AX)
                        stdq = s2.tile([P, NG], f32, tag="hstd", bufs=3)
                        nc.scalar.activation(stdq[:, :gw], sumsq[:, :gw],
                                             AF.Sqrt, bias=epsC,
                                             scale=1.0 / HD)
                        rstd = s2.tile([P, NG], f32, tag="hrstd", bufs=3)
                        nc.vector.reciprocal(rstd[:, :gw], stdq[:, :gw])
                        zv = zbs[s][:, f0:f0 + C].rearrange(
                            "p (g d) -> p g d", d=HD)
                        nc.vector.tensor_tensor(
                            zv,
                            acc.rearrange("p (g d) -> p g d", d=HD),
                            rstd[:, :gw, None].to_broadcast([P, gw, HD]),
                            ALU.mult)
                        if hln_aff:
                            nc.vector.tensor_tensor(
                                zv, zv,
                                hlnw_sb[:, None, :].to_broadcast([P, gw, HD]),
                                ALU.mult)
                            nc.vector.tensor_tensor(
                                zv, zv,
                                hlnb_sb[:, None, :].to_broadcast([P, gw, HD]),
                                ALU.add)

                def sect_tp(s, half):
                    with nc.named_scope(f"qkv_{s}"):
                        tp2 = ps.tile([P, KC, P], bf16, tag="A", name="tp2")
                        for t in range(KC):
                            tt = half * KC + t
                            nc.tensor.transpose(
                                tp2[:, t, :], zbs[s][:, tt * P:(tt + 1) * P],
                                ident)
                        qkt_sb = s2.tile([P, KC, P], bf16, tag="qkt", bufs=3)
                        nc.scalar.copy(qkt_sb, tp2)
                        nc.sync.dma_start(
                            qkT_t[s][:].rearrange("(t p) n -> p t n", p=P)
                            [:, half * KC:(half + 1) * KC, cs],
                            qkt_sb)

                acc_q = {s: sect_mm(s, 0) for s in "ba"}
                acc_k = {}
                for s in "ba":
                    acc_k[s] = sect_mm(s, 1)
                for s in "ba":
                    sect_norm(s, 0, acc_q[s])
                acc_v = {}
                for s in "ba":
                    acc_v[s] = sect_mm(s, 2)
                for s in "ba":
                    sect_norm(s, 1, acc_k[s])
                for s in "ba":
                    sect_tp(s, 1)            # k transposes first for attn
                for s in "ba":
                    sect_norm(s, 2, acc_v[s])
                for s in "ba":
                    # stores depend only on norms -- issue before q transposes
                    # so attention's va/qr loads unblock sooner
                    with nc.named_scope(f"qkv_{s}"):
                        nc.gpsimd.dma_start(v_t[s][cs, :], zbs[s][:, 2 * C:])
                        nc.sync.dma_start(
                            qr_t[s][:].rearrange("(h n) d -> n h d", h=H)[cs],
                            zbs[s][:, :C].rearrange("p (g d) -> p g d", d=HD))
                for s in "ba":
                    sect_tp(s, 0)

            for s in "ba":
                p1_ln(s, 0)
            for c in range(NCH):
                for s in "ba":
                    p1_tp(s, c)
                if c + 1 < NCH:
                    for s in "ba":
                        p1_ln(s, c + 1)
                qkv_chunk2(c)

            # ======== attention + proj + mlp, software-pipelined ========
            DIRS = (("b", "a"), ("a", "b"))  # (qs, ks); output goes to stream ks
            ctx2 = {}
            for qs, ks in DIRS:
                ctx2[qs] = big.tile([P, NJ, N], bf16, name=f"ctx2_{qs}", tag="big")
            # persistent [v | ones] stationaries: parity x direction
            vap = {}
            for hp in range(2):
                for qs, ks in DIRS:
                    t = s1.tile([P, NCH, P], f8, tag=f"va{hp}{qs}")
                    nc.gpsimd.memset(t[:, :, (1 - hp) * HD:(2 - hp) * HD], 1.0)
                    vap[(hp, qs)] = t

            # prefetch mlp fc2 weights (shares the wq slot; frees at attn start)
            w2sb = s1.tile([P, HKC, C], f8, tag="wbig")
            nc.gpsimd.dma_start(w2sb, w2p_d[:].rearrange("p (k o) -> p k o", o=C))
            w1v = w1p_d[:].rearrange("p (kc k f) -> p kc k f", k=KC, f=P)

            def attn_head(j, hp):
                """Both directions merged: QK bursts of 8 back-to-back
                matmuls (>3us: PE ramps to max p-state); PV deferred one
                mc2 so it never waits on exp."""
                h = 2 * j + hp
                hs = slice(hp * HD, (hp + 1) * HD)        # ctx half
                ds = slice((1 - hp) * HD, (2 - hp) * HD)  # denominator half
                lo = slice(0, HD)
                qts, kts, cps = {}, {}, {}
                for qs, ks in DIRS:
                    with nc.named_scope(f"attn_{qs}"):
                        qt = s3b.tile([HD, N], bf16, tag="qh", name=f"qh_{qs}{h}")
                        nc.sync.dma_start(qt, qkT_t[qs][h * HD:(h + 1) * HD, :])
                        kt = s3b.tile([HD, N], bf16, tag="kh", name=f"kh_{qs}{h}")
                        nc.sync.dma_start(
                            kt, qkT_t[ks][C + h * HD:C + (h + 1) * HD, :])
                        va = vap[(hp, qs)]
                        nc.sync.dma_start(
                            va[:, :, hp * HD:(hp + 1) * HD],
                            v_t[ks][:].rearrange("(c p) f -> p c f", p=P)
                            [:, :, h * HD:(h + 1) * HD])
                        qts[qs], kts[qs] = qt, kt
                        cps[qs] = ps.tile([P, 2, 512], f32, tag="A",
                                          name=f"cps_{qs}")
                pend = []  # deferred PV: (qs, mc2, pt2)
                for mc2 in range(NCH // 2):
                    cur = []
                    for qs, ks in DIRS:
                        with nc.named_scope(f"attn_{qs}"):
                            # pt2[p, nh, mi, n]: PV k-tile pair contiguous
                            pt2 = s3b.tile([P, 2, 2, 512], f8, tag="pt",
                                           bufs=4)
                            for mi in range(2):
                                mc = 2 * mc2 + mi
                                sps = ps.tile([P, 2, 512], f32, tag="A",
                                              name="sps")
                                for nh in range(2):
                                    nc.tensor.matmul(
                                        sps[:, nh, :],
                                        kts[qs][:, mc * P:(mc + 1) * P],
                                        qts[qs][:, nh * 512:(nh + 1) * 512])
                                nc.scalar.activation(
                                    pt2[:, :, mi, :], sps,
                                    AF.Exp, scale=float(HD ** -0.5), bias=eln)
                            cur.append((qs, mc2, pt2))
                    for qs, pmc2, pt2 in pend:
                        with nc.named_scope(f"attn_{qs}"):
                            for nh in range(2):
                                nc.tensor.matmul(
                                    cps[qs][:, nh, :],
                                    vap[(hp, qs)][:, 2 * pmc2:2 * pmc2 + 2, :],
                                    pt2[:, nh],
                                    perf_mode=DR,
                                    start=(pmc2 == 0),
                                    stop=(pmc2 == NCH // 2 - 1))
                    pend = cur
                for qs, pmc2, pt2 in pend:
                    with nc.named_scope(f"attn_{qs}"):
                        for nh in range(2):
                            nc.tensor.matmul(
                                cps[qs][:, nh, :],
                                vap[(hp, qs)][:, 2 * pmc2:2 * pmc2 + 2, :],
                                pt2[:, nh],
                                perf_mode=DR,
                                start=(pmc2 == 0),
                                stop=(pmc2 == NCH // 2 - 1))
                for qs, ks in DIRS:
                    with nc.named_scope(f"attn_{qs}"):
                        # denominator (replicated on partitions ds):
                        # aligned copy out of PSUM, shift to base 0,
                        # recipfast at base 0, shift to hs, aligned mult.
                        dn = s3.tile([P, N], f32, tag="dn")
                        nc.vector.tensor_copy(
                            dn[ds, :],
                            cps[qs][ds, :, :].rearrange("p a b -> p (a b)"))
                        if hp == 0:
                            nc.gpsimd.dma_start(dn[lo, :], dn[ds, :])
                        rd = s3.tile([P, N], f32, tag="rd")
                        nc.vector.reciprocal_approx_fast(rd[lo, :], dn[lo, :])
                        if hp == 1:
                            nc.gpsimd.dma_start(rd[hs, :], rd[lo, :])
                        nc.vector.tensor_tensor(
                            ctx2[qs][hs, j, :],
                            cps[qs][hs, :, :].rearrange("p a b -> p (a b)"),
                            rd[hs, :], ALU.mult)

            def proj_mm(qs, ks, c):
                """proj matmul + residual + LN2 stats for chunk c."""
                cs = slice(c * P, (c + 1) * P)
                with nc.named_scope(f"proj_{ks}"):
                    qr_view = qr_t[qs][:].rearrange("(n j) d -> n (j d)", j=H)
                    xnr = s2.tile([P, C], bf16, tag="xnr", bufs=3)
                    nc.sync.dma_start(xnr, xn_t[ks][cs, :])
                    qres = s2.tile([P, C], bf16, tag="qres", bufs=3)
                    nc.sync.dma_start(qres, qr_view[cs, :])
                    y = ps.tile([P, C], f32, tag="A", name="yproj")
                    for jj in range(NJ):
                        for o0, ow in ((0, 512), (512, 256)):
                            nc.tensor.matmul(
                                y[:, o0:o0 + ow],
                                ctx2[qs][:, jj, cs],
                                pw2sb[:, jj, o0:o0 + ow],
                                start=(jj == 0), stop=False)
                    # residual adds ride the PSUM accumulation as
                    # identity-stationary matmuls (vector was the pacer here)
                    for res, last in ((xnr, False), (qres, True)):
                        for o0, ow in ((0, 512), (512, 256)):
                            nc.tensor.matmul(
                                y[:, o0:o0 + ow], ident,
                                res[:, o0:o0 + ow],
                                start=False,
                                stop=(last and ow == 256 and not has_projb))
                    if has_projb:
                        for o0, ow in ((0, 512), (512, 256)):
                            nc.tensor.matmul(
                                y[:, o0:o0 + ow], ones_bf[0:1, :],
                                projb_sb[0:1, o0:o0 + ow],
                                start=False, stop=(ow == 256))
                    ot = s2.tile([P, C], bf16, tag="otb", bufs=3)
                    nc.scalar.copy(ot, y)
                    nc.sync.dma_start(o_t[ks][cs, :], ot)
                    mv, rstd = ln_stats(ot)
                    return (ks, c, ot, mv, rstd)

            def proj_tail(state):
                """LN2 apply + transpose + store, one chunk behind proj_mm."""
                ks, c, ot, mv, rstd = state
                cs = slice(c * P, (c + 1) * P)
                with nc.named_scope(f"proj_{ks}"):
                    x2b = s2.tile([P, C], bf16, tag="x2b", bufs=3)
                    ln_apply(x2b, ot, mv, rstd,
                             n2w_sb if n2_aff else None,
                             n2b_sb if n2_aff else None)
                    tp3 = ps.tile([P, KC, P], bf16, tag="A", name="tp3")
                    for t in range(KC):
                        nc.tensor.transpose(tp3[:, t, :], x2b[:, t * P:(t + 1) * P],
                                            ident)
                    x2ts = s2.tile([P, KC, P], bf16, tag="x2ts", bufs=3)
                    nc.scalar.copy(x2ts, tp3)
                    nc.sync.dma_start(
                        x2T_t[ks][:].rearrange("(t p) n -> p t n", p=P)[:, :, cs],
                        x2ts)

            # MLP state per stream
            mlp_x2h = {}
            mlp_hT = {}

            def mlp_load(s):
                with nc.named_scope(f"mlp_{s}"):
                    x2h = []
                    for nh in range(2):
                        xh = sB.tile([P, KC, 512], f8, tag="x2h", bufs=4,
                                     name=f"x2h_{s}{nh}")
                        nc.gpsimd.dma_start(
                            out=xh,
                            in_=x2T_t[s][:].rearrange("(k p) n -> p k n", p=P)
                            [:, :, nh * 512:(nh + 1) * 512])
                        x2h.append(xh)
                    mlp_x2h[s] = x2h
                    hT = []
                    for nh in range(2):
                        hT.append(big.tile([P, HKC, 512], f8, tag="big",
                                           name=f"hT_{s}{nh}"))
                    mlp_hT[s] = hT

            def fc1_step(kc2, streams="ab"):
                """fc1 at kc2 for given streams."""
                w1k = []
                for kk in range(2):
                    w1t = s3b.tile([P, KC, P], f8, tag="w1k", bufs=3)
                    nc.sync.dma_start(w1t, w1v[:, 2 * kc2 + kk, :, :])
                    w1k.append(w1t)
                for s in streams:
                    x2h = mlp_x2h[s]
                    hT = mlp_hT[s]
                    with nc.named_scope(f"mlp_{s}"):
                        # kk-major: gelu(kk=0) overlaps the kk=1 matmuls
                        accs = [ps.tile([P, 2, 512], f32, tag="A",
                                        name=f"facc{nh}")
                                for nh in range(2)]
                        for kk in range(2):
                            for kp in range(0, KC, 2):
                                for nh in range(2):
                                    nc.tensor.matmul(
                                        accs[nh][:, kk, :],
                                        w1k[kk][:, kp:kp + 2, :],
                                        x2h[nh][:, kp:kp + 2, :],
                                        perf_mode=DR,
                                        start=(kp == 0), stop=(kp == KC - 2))
                            kc = 2 * kc2 + kk
                            for nh in range(2):
                                if has_fc1b:
                                    nc.scalar.activation(
                                        hT[nh][:, kc, :], accs[nh][:, kk, :],
                                        AF.Gelu, bias=fc1b_sb[:, kc:kc + 1],
                                        scale=1.0 / W8SCALE)
                                else:
                                    nc.scalar.activation(
                                        hT[nh][:, kc, :], accs[nh][:, kk, :],
                                        AF.Gelu, scale=1.0 / W8SCALE)

            def fc2_chunk(s, nh, sub):
                hT = mlp_hT[s]
                c = nh * 4 + sub
                cs = slice(c * P, (c + 1) * P)
                with nc.named_scope(f"mlp_{s}"):
                    y = ps.tile([P, C], f32, tag="A", name="yfc2")
                    for kc in range(0, HKC, 2):
                        for o0, ow in ((0, 512), (512, 256)):
                            nc.tensor.matmul(
                                y[:, o0:o0 + ow],
                                hT[nh][:, kc:kc + 2, sub * P:(sub + 1) * P],
                                w2sb[:, kc:kc + 2, o0:o0 + ow],
                                perf_mode=DR,
                                start=(kc == 0),
                                stop=(kc == HKC - 2 and not has_fc2b))
                    if has_fc2b:
                        for o0, ow in ((0, 512), (512, 256)):
                            nc.tensor.matmul(
                                y[:, o0:o0 + ow], ones_bf[0:1, :],
                                fc2b_sb[0:1, o0:o0 + ow],
                                start=False, stop=True)
                    oh = s2.tile([P, C], bf16, tag="oh")
                    nc.sync.dma_start(oh, o_t[s][cs, :])
                    outt = sF.tile([P, C], f32, tag="f32buf", name="outt")
                    nc.vector.scalar_tensor_tensor(
                        outt, y, 1.0 / W8SCALE, oh, ALU.mult, ALU.add)
                    nc.sync.dma_start(out_d[s][cs, :], outt)

            # ---- attention: both dirs merged per head ----
            for j in range(NJ):
                for hp in range(2):
                    attn_head(j, hp)

            # ---- proj both streams, alternating chunks, tail skewed ----
            prev = []
            for c in range(NCH):
                cur = [proj_mm("b", "a", c), proj_mm("a", "b", c)]
                if c == NCH - 1:
                    mlp_load("a")    # x2T_a complete after this proj_mm pair
                for st in prev:
                    proj_tail(st)
                prev = cur
            for st in prev:
                proj_tail(st)
            mlp_load("b")
            for kc2 in range(HKC // 2):
                fc1_step(kc2, "a")
            FC2A = {0: [0], 1: [1], 2: [2], 3: [3], 4: [4], 5: [5],
                    6: [6], 7: [7], 8: [], 9: [], 10: [], 11: []}
            for kc2 in range(HKC // 2):
                fc1_step(kc2, "b")
                for idx in FC2A.get(kc2, []):
                    fc2_chunk("a", idx // 4, idx % 4)
            for idx in range(8):
                fc2_chunk("b", idx // 4, idx % 4)

    nc.finalize()
    return nc


def _get_nc(flags):
    if flags not in _CACHE:
        _CACHE[flags] = _build(flags)
    return _CACHE[flags]


def _prep(inputs):
    import ml_dtypes

    f = np.float32
    bf = ml_dtypes.bfloat16
    f8 = ml_dtypes.float8_e4m3
    w = {k: np.asarray(v, f) for k, v in inputs.items()}
    flags = (
        not (np.all(w["norm1_w"] == 1) and np.all(w["norm1_b"] == 0)),
        not (np.all(w["hln_w"] == 1) and np.all(w["hln_b"] == 0)),
        not (np.all(w["norm2_w"] == 1) and np.all(w["norm2_b"] == 0)),
        bool(np.any(w["proj_b"] != 0)),
        bool(np.any(w["fc1_b"] != 0)),
        bool(np.any(w["fc2_b"] != 0)),
    )
    # qkv weights: transpose + fold head-LN centering (linear in x)
    wT = np.ascontiguousarray(w["qkv_w"].T)                   # [C, 3C]
    wT3 = wT.reshape(C, NG, HD)
    wTc = (wT3 - wT3.mean(axis=2, keepdims=True)).reshape(C, S3)
    # proj weights packed by head pair: pw2[p=(h%2)*64+d, j=h//2, o]
    pw = w["proj_w"].T.reshape(NJ, 2, HD, C).transpose(1, 2, 0, 3).reshape(P, NJ * C)
    # fc1 packed: w1p[p, kc, k, f'] = 32*fc1_w[kc*128+f', k*128+p], fp8
    w1p = (W8SCALE * w["fc1_w"]).reshape(HKC, P, KC, P).transpose(3, 0, 2, 1).reshape(P, HKC * C)
    # fc2 packed: w2p[p, kc, o] = 32*fc2_w[o, kc*128+p], fp8
    w2p = (W8SCALE * w["fc2_w"]).reshape(C, HKC, P).transpose(2, 1, 0).reshape(P, HKC * C)
    shared = {
        "qkv_wT": wTc.astype(bf),
        "pw2": np.ascontiguousarray(pw).astype(bf),
        "w1p": np.ascontiguousarray(w1p).astype(f8),
        "w2p": np.ascontiguousarray(w2p).astype(f8),
    }
    n1_aff, hln_aff, n2_aff, pb, f1b, f2b = flags
    if pb:
        shared["proj_b"] = w["proj_b"].reshape(1, C).astype(bf)
    if f1b:
        shared["fc1_b"] = w["fc1_b"]
    if f2b:
        shared["fc2_b"] = (w["fc2_b"] * W8SCALE).reshape(1, C).astype(bf)
    if n1_aff:
        shared["norm1_w"] = w["norm1_w"]
        shared["norm1_b"] = w["norm1_b"]
    if n2_aff:
        shared["norm2_w"] = w["norm2_w"]
        shared["norm2_b"] = w["norm2_b"]
    if hln_aff:
        shared["hln_w"] = w["hln_w"]
        shared["hln_b"] = w["hln_b"]
    return w, flags, shared


def kernel(trace=False, **inputs):
    from concourse.bass_utils import run_bass_kernel_spmd

    w, flags, shared = _prep(inputs)
    nc = _get_nc(flags)
    before = np.ascontiguousarray(w["before"], dtype=np.float32)
    after = np.ascontiguousarray(w["after"], dtype=np.float32)
    in_maps = []
    for core in range(B):
        m = dict(shared)
        m["x_b"] = np.ascontiguousarray(before[core])
        m["x_a"] = np.ascontiguousarray(after[core])
        in_maps.append(m)
    res = run_bass_kernel_spmd(nc, in_maps, core_ids=list(range(B)), trace=trace)
    before_o = np.stack([res.results[i]["out_b"] for i in range(B)])
    after_o = np.stack([res.results[i]["out_a"] for i in range(B)])
    out = (before_o.astype(np.float32), after_o.astype(np.float32))
    if trace:
        return out, res
    return out
